# revision 1
# baseline (speedup 1.0000x reference)
"""Trainium2 Bass kernel for nn_CrossAttention (B=16, SQ=1, SKV=4096, D=1024, H=16).

Strategy
--------
Data-parallel over batch: each of the 8 cores owns 2 batch elements.

The naive computation projects all of K and V (two [4096,1024]x[1024,1024]
GEMMs per batch = 275 GFLOP total).  Because SQ == 1, we restructure:

  scores^T[h,kpos] = sum_j t[h,j] * key[kpos,j],  t[h,:] = SCALE * qh[h] @ Wk_h
    (the bk bias is constant along kpos, so it cancels in softmax)
  wv[h,j]   = sum_kpos e[h,kpos] * value[kpos,j]   (raw value, project after)
  attn[h,:] = (wv[h,:]/S) @ Wv_h^T + bv_h          (sum of softmax weights = 1)

which drops compute to ~4.3 GFLOP and makes the kernel DMA-bound on
streaming K and V once.  K/V (and e) are cast to bf16 on the host to halve
the DMA bytes; weights and everything else stay fp32 (measured end-to-end
error vs the fp32 reference: ~6e-4 Frobenius-relative).

Per-core pipeline (b = 0,1 local batches):
  q^T = (query @ Wq^T + bq) * SCALE          (f32r matmuls + PE transpose)
  t   = blockdiag(q^T) @ Wk                  (f32r; masked-qT trick batches heads)
  per b: scores^T = tT^T @ keyT (bf16)  -> softmax (max, exp w/ fused sum)
         e^T tiles (PE transpose) ; wv = e^T^T @ value (bf16)
         wv/S -> wv^T tiles ; attnT = WvT^T @ wvT ; diagonal-block extract
  out = attn_flat @ Wo^T + bo                (f32r)
"""

import numpy as np
import ml_dtypes
from contextlib import ExitStack

import concourse.bass as bass
from concourse import bacc
import concourse.mybir as mybir
from concourse.tile import TileContext
from concourse.bass_utils import run_bass_kernel_spmd

B, SKV, D, H, HD = 16, 4096, 1024, 16, 64
NCORES = 8
BPC = B // NCORES  # 2 batches per core
SCALE = 1.0 / float(D) ** 0.5

FP32 = mybir.dt.float32
F32R = mybir.dt.float32r
BF16 = mybir.dt.bfloat16
AX = mybir.AxisListType.X
EXP = mybir.ActivationFunctionType.Exp
IDENT = mybir.ActivationFunctionType.Identity

BF = np.dtype(ml_dtypes.bfloat16)

_CACHE = {}


def r(ap):
    # float32r needs a rounding producer; plain fp32 matmul is fine here
    return ap


def build_nc():
    nc = bacc.Bacc("TRN2")

    # ---- kernel parameters (per core) ----
    queryT = nc.declare_dram_parameter("queryT", [D, BPC], FP32, isOutput=False)
    keyT = nc.declare_dram_parameter("keyT", [BPC, D, SKV], BF16, isOutput=False)
    value = nc.declare_dram_parameter("value", [BPC, SKV, D], BF16, isOutput=False)
    WqT = nc.declare_dram_parameter("WqT", [D, D], FP32, isOutput=False)
    Wk = nc.declare_dram_parameter("Wk", [D, D], FP32, isOutput=False)
    WvT = nc.declare_dram_parameter("WvT", [D, D], FP32, isOutput=False)
    WoT = nc.declare_dram_parameter("WoT", [D, D], FP32, isOutput=False)
    bqsT = nc.declare_dram_parameter("bqsT", [128, 8], FP32, isOutput=False)
    bvT = nc.declare_dram_parameter("bvT", [128, 8], FP32, isOutput=False)
    bo2 = nc.declare_dram_parameter("bo2", [BPC, D], FP32, isOutput=False)
    id32 = nc.declare_dram_parameter("id32", [32, 32], FP32, isOutput=False)
    idbf = nc.declare_dram_parameter("idbf", [16, 16], BF16, isOutput=False)
    out_ext = nc.declare_dram_parameter("out", [BPC, D], FP32, isOutput=True)

    # [p, n, x] views of the big weight matrices (row r = n*128 + p)
    WqT_r = WqT.rearrange("(n p) o -> p n o", p=128)
    Wk_r = Wk.rearrange("(n p) j -> p n j", p=128)
    WvT_r = WvT.rearrange("(n p) o -> p n o", p=128)
    WoT_r = WoT.rearrange("(n p) o -> p n o", p=128)
    queryT_r = queryT.rearrange("(n p) b -> p n b", p=128)

    with TileContext(nc) as tc, ExitStack() as ctx:
        consts = ctx.enter_context(tc.tile_pool(name="consts", bufs=1))
        wqk = ctx.enter_context(tc.tile_pool(name="wqk", bufs=2))
        keyp = ctx.enter_context(tc.tile_pool(name="keyp", bufs=4))
        valp = ctx.enter_context(tc.tile_pool(name="valp", bufs=4))
        sbig = ctx.enter_context(tc.tile_pool(name="sbig", bufs=2))
        small = ctx.enter_context(tc.tile_pool(name="small", bufs=2))
        elp = ctx.enter_context(tc.tile_pool(name="elp", bufs=6))
        ps_sc = ctx.enter_context(tc.tile_pool(name="ps_sc", bufs=2, space="PSUM"))
        ps_wv = ctx.enter_context(tc.tile_pool(name="ps_wv", bufs=2, space="PSUM"))
        ps_m = ctx.enter_context(tc.tile_pool(name="ps_m", bufs=2, space="PSUM"))

        # ---- resident constants ----
        wvt_sb = consts.tile([128, 8, D], FP32, tag="wvt")
        nc.sync.dma_start(out=wvt_sb, in_=WvT_r)
        wot_sb = consts.tile([128, 8, D], FP32, tag="wot")
        nc.sync.dma_start(out=wot_sb, in_=WoT_r)
        bqs_sb = consts.tile([128, 8], FP32, tag="bqs")
        nc.sync.dma_start(out=bqs_sb, in_=bqsT[:, :])
        bvt_sb = consts.tile([128, 8], FP32, tag="bvt")
        nc.sync.dma_start(out=bvt_sb, in_=bvT[:, :])
        bo_sb = consts.tile([BPC, D], FP32, tag="bo")
        nc.sync.dma_start(out=bo_sb, in_=bo2[:, :])
        id32_sb = consts.tile([32, 32], FP32, tag="id32")
        nc.sync.dma_start(out=id32_sb, in_=id32[:, :])
        idbf_sb = consts.tile([16, 16], BF16, tag="idbf")
        nc.sync.dma_start(out=idbf_sb, in_=idbf[:, :])
        qin_sb = consts.tile([128, 8, BPC], FP32, tag="qin")
        nc.sync.dma_start(out=qin_sb, in_=queryT_r)

        # ---- q^T = (query @ Wq^T + bq) * SCALE ----
        q_ps = [ps_m.tile([BPC, 512], FP32, tag="m", name=f"q_ps{i}") for i in range(2)]
        for ic in range(8):
            wt = wqk.tile([128, D], FP32, tag="w", name="wq")
            nc.sync.dma_start(out=wt, in_=WqT_r[:, ic, :])
            for half in range(2):
                nc.tensor.matmul(
                    q_ps[half],
                    r(qin_sb[:, ic, :]),
                    r(wt[:, half * 512 : (half + 1) * 512]),
                    start=(ic == 0),
                    stop=(ic == 7),
                )
        q_sb = small.tile([BPC, D], FP32, tag="q", bufs=1)
        for half in range(2):
            nc.vector.tensor_copy(q_sb[:, half * 512 : (half + 1) * 512], q_ps[half])
        # transpose to qT [128, 8(ot), BPC] with scale+bias fused
        qt_sb = consts.tile([128, 8, BPC], FP32, tag="qt")
        for ot in range(8):
            tp = ps_m.tile([128, BPC], FP32, tag="m")
            nc.tensor.transpose(tp, q_sb[:, ot * 128 : (ot + 1) * 128], id32_sb[:BPC, :BPC])
            nc.scalar.activation(
                out=qt_sb[:, ot, :], in_=tp, func=IDENT,
                bias=bqs_sb[:, ot : ot + 1], scale=SCALE,
            )

        # ---- t = blockdiag(qT) @ Wk : col (h*2+b) holds qT rows of head h ----
        qmask_sb = consts.tile([128, 8, 32], FP32, tag="qmask")
        nc.vector.memset(qmask_sb, 0.0)
        for ic in range(8):
            for b in range(BPC):
                nc.vector.tensor_copy(
                    qmask_sb[0:64, ic, 4 * ic + b : 4 * ic + b + 1],
                    qt_sb[0:64, ic, b : b + 1],
                )
                nc.vector.tensor_copy(
                    qmask_sb[64:128, ic, 4 * ic + 2 + b : 4 * ic + 3 + b],
                    qt_sb[64:128, ic, b : b + 1],
                )
        t_ps = [ps_m.tile([32, 512], FP32, tag="m", name=f"t_ps{i}") for i in range(2)]
        for ic in range(8):
            wt = wqk.tile([128, D], FP32, tag="w", name="wk")
            nc.sync.dma_start(out=wt, in_=Wk_r[:, ic, :])
            for half in range(2):
                nc.tensor.matmul(
                    t_ps[half],
                    r(qmask_sb[:, ic, :]),
                    r(wt[:, half * 512 : (half + 1) * 512]),
                    start=(ic == 0),
                    stop=(ic == 7),
                )
        t_sb = small.tile([32, D], FP32, tag="t", bufs=1)
        for half in range(2):
            nc.vector.tensor_copy(t_sb[:, half * 512 : (half + 1) * 512], t_ps[half])
        # transpose: tT[jc] [128(j), 32(h,b)] -> per-b bf16 [128, 16]
        tT = [[None] * 8 for _ in range(BPC)]
        for jc in range(8):
            tp = ps_m.tile([128, 32], FP32, tag="m")
            nc.tensor.transpose(tp, t_sb[:, jc * 128 : (jc + 1) * 128], id32_sb)
            tp_v = tp.rearrange("p (h b) -> p b h", b=BPC)
            for b in range(BPC):
                tt = consts.tile([128, 16], BF16, tag=f"tT{jc}_{b}", name=f"tT{jc}_{b}")
                nc.vector.tensor_copy(tt, tp_v[:, b, :])
                tT[b][jc] = tt

        # ---- per-batch attention ----
        attn_lhsT = [
            consts.tile([128, BPC], FP32, tag=f"al{t2}", name=f"al{t2}")
            for t2 in range(8)
        ]
        for b in range(BPC):
            scores_sb = sbig.tile([16, SKV], BF16, tag="scores", bufs=2)
            e_sb = sbig.tile([16, SKV], BF16, tag="e")
            keyT_r = keyT[b].rearrange("(n p) s -> p n s", p=128)
            cmax = small.tile([16, 8], FP32, tag="cmax")
            for kt in range(8):
                # column-block of keyT: all 8 j-chunks for 512 kpos
                kt_sb = keyp.tile([128, 8, 512], BF16, tag="k", name="kt_sb")
                nc.sync.dma_start(out=kt_sb, in_=keyT_r[:, :, kt * 512 : (kt + 1) * 512])
                sc_ps = ps_sc.tile([16, 512], FP32, tag="sc")
                for jc in range(8):
                    nc.tensor.matmul(
                        sc_ps,
                        tT[b][jc],
                        kt_sb[:, jc, :],
                        start=(jc == 0),
                        stop=(jc == 7),
                    )
                nc.scalar.copy(scores_sb[:, kt * 512 : (kt + 1) * 512], sc_ps)
                nc.vector.reduce_max(out=cmax[:, kt : kt + 1], in_=sc_ps, axis=AX)
            # softmax (unnormalized) with fused sum-of-exp
            m1 = small.tile([16, 1], FP32, tag="m1")
            nc.vector.reduce_max(out=m1, in_=cmax, axis=AX)
            negm = small.tile([16, 1], FP32, tag="negm")
            nc.vector.tensor_scalar_mul(negm, m1, -1.0)
            S = small.tile([16, 1], FP32, tag="S")
            nc.scalar.activation(
                out=e_sb, in_=scores_sb,
                func=EXP, bias=negm, scale=1.0, accum_out=S,
            )
            rS = small.tile([16, 1], FP32, tag="rS")
            nc.vector.reciprocal(rS, S)

            # stream value[b]; e^T tiles; wv accumulation
            val_r = value[b].rearrange("(c p) j -> p c j", p=128)
            el = []
            for kt in range(32):
                tp = ps_m.tile([128, 16], BF16, tag="m")
                nc.tensor.transpose(
                    tp, e_sb[:, kt * 128 : (kt + 1) * 128], idbf_sb
                )
                e1 = elp.tile([128, 16], BF16, tag="el")
                nc.vector.tensor_copy(e1, tp)
                el.append(e1)
            wv_ps = ps_wv.tile([16, D], FP32, tag="wv")
            for ti in range(8):
                vt = valp.tile([128, 4, D], BF16, tag="v", name="vt")
                nc.sync.dma_start(out=vt, in_=val_r[:, 4 * ti : 4 * ti + 4, :])
                for c in range(4):
                    kt = ti * 4 + c
                    for half in range(2):
                        nc.tensor.matmul(
                            wv_ps[:, half * 512 : (half + 1) * 512],
                            el[kt],
                            vt[:, c, half * 512 : (half + 1) * 512],
                            start=(kt == 0),
                            stop=(kt == 31),
                        )
            # wv/S -> fp32, transpose to wvT tiles
            wv_sb = small.tile([16, D], FP32, tag="wvsb")
            nc.scalar.activation(out=wv_sb, in_=wv_ps, func=IDENT, bias=0.0, scale=rS)
            wvT = []
            for jc in range(8):
                tp = ps_m.tile([128, 16], FP32, tag="m")
                nc.tensor.transpose(
                    tp, wv_sb[:, jc * 128 : (jc + 1) * 128], id32_sb[:16, :16]
                )
                w1 = elp.tile([128, 16], FP32, tag="wvT", bufs=8, name="w1")
                nc.vector.tensor_copy(w1, tp)
                wvT.append(w1)
            # attnT[t2] [128(hd), 16(h)] = sum_jc WvT[jc,t2]^T . wvT[jc]
            for t2 in range(8):
                at_ps = ps_m.tile([128, 16], FP32, tag="m")
                for jc in range(8):
                    nc.tensor.matmul(
                        at_ps,
                        wvt_sb[:, jc, t2 * 128 : (t2 + 1) * 128],
                        wvT[jc],
                        start=(jc == 0),
                        stop=(jc == 7),
                    )
                # diagonal-block extract: heads (2*t2, 2*t2+1)
                nc.vector.tensor_copy(
                    attn_lhsT[t2][0:64, b : b + 1], at_ps[0:64, 2 * t2 : 2 * t2 + 1]
                )
                nc.vector.tensor_copy(
                    attn_lhsT[t2][64:128, b : b + 1],
                    at_ps[64:128, 2 * t2 + 1 : 2 * t2 + 2],
                )

        # ---- bv bias, final projection, output ----
        for t2 in range(8):
            nc.scalar.activation(
                out=attn_lhsT[t2], in_=attn_lhsT[t2], func=IDENT,
                bias=bvt_sb[:, t2 : t2 + 1], scale=1.0,
            )
        out_sb = small.tile([BPC, D], FP32, tag="out", bufs=1)
        for half in range(2):
            o_ps = ps_m.tile([BPC, 512], FP32, tag="m")
            for t2 in range(8):
                nc.tensor.matmul(
                    o_ps,
                    r(attn_lhsT[t2]),
                    r(wot_sb[:, t2, half * 512 : (half + 1) * 512]),
                    start=(t2 == 0),
                    stop=(t2 == 7),
                )
            nc.vector.tensor_add(
                out_sb[:, half * 512 : (half + 1) * 512],
                o_ps,
                bo_sb[:, half * 512 : (half + 1) * 512],
            )
        nc.sync.dma_start(out=out_ext[:, :], in_=out_sb)

    if not nc.is_finalized():
        nc.finalize()
    return nc


def _prep_in_maps(inputs):
    query = np.asarray(inputs["query"], np.float32)
    key = np.asarray(inputs["key"], np.float32)
    value = np.asarray(inputs["value"], np.float32)
    Wq = np.asarray(inputs["Wq"], np.float32)
    bq = np.asarray(inputs["bq"], np.float32)
    Wk = np.asarray(inputs["Wk"], np.float32)
    Wv = np.asarray(inputs["Wv"], np.float32)
    Wo = np.asarray(inputs["Wo"], np.float32)
    bv = np.asarray(inputs["bv"], np.float32)
    bo = np.asarray(inputs["bo"], np.float32)

    shared = {
        "WqT": np.ascontiguousarray(Wq.T),
        "Wk": np.ascontiguousarray(Wk),
        "WvT": np.ascontiguousarray(Wv.T),
        "WoT": np.ascontiguousarray(Wo.T),
        "bqsT": np.ascontiguousarray((bq * SCALE).reshape(8, 128).T),
        "bvT": np.ascontiguousarray(bv.reshape(8, 128).T),
        "bo2": np.ascontiguousarray(np.broadcast_to(bo, (BPC, D))),
        "id32": np.eye(32, dtype=np.float32),
        "idbf": np.eye(16, dtype=np.float32).astype(BF),
    }
    in_maps = []
    for c in range(NCORES):
        c0 = c * BPC
        in_maps.append(
            {
                "queryT": np.ascontiguousarray(query[c0 : c0 + BPC, 0, :].T),
                "keyT": np.ascontiguousarray(
                    key[c0 : c0 + BPC].transpose(0, 2, 1)
                ).astype(BF),
                "value": np.ascontiguousarray(value[c0 : c0 + BPC]).astype(BF),
                **shared,
            }
        )
    return in_maps


def kernel(**inputs):
    if "nc" not in _CACHE:
        _CACHE["nc"] = build_nc()
    nc = _CACHE["nc"]
    in_maps = _prep_in_maps(inputs)
    res = run_bass_kernel_spmd(nc, in_maps, list(range(NCORES)))
    return np.concatenate([res.results[i]["out"] for i in range(NCORES)], axis=0)


if __name__ == "__main__":
    nc = build_nc()
    print("built ok")



# revision 6
# speedup vs baseline: 2.3242x; 2.3242x over previous
"""Trainium2 Bass kernel for nn_CrossAttention (B=16, SQ=1, SKV=4096, D=1024, H=16).

Strategy
--------
Data-parallel over batch: each of the 8 cores owns 2 batch elements.

Because SQ == 1 the K/V projections fold away:

  scores[h,kpos] = t[h,:] . key[kpos,:],   t = SCALE * blockdiag(qh) @ Wk
    (bk is constant along kpos -> cancels in softmax)
  wv[h,:]   = e[h,:] @ value            (raw value, project after)
  attn[h,:] = (wv[h,:]/S) @ Wv_h^T      (+ bv folds into bo on host: Sum w = 1)

This drops compute ~64x vs the naive form and makes the kernel DMA-bound
on streaming K and V once.  K/V (and Wq/Wk, x64-scaled to dodge e3m4
subnormals) are cast to fp8 e3m4 on the host, halving DMA vs bf16;
Wv/Wo stay bf16 (fp8 there is too lossy).  Measured end-to-end error
vs the fp32 reference: ~4e-3 Frobenius-relative.

All matmuls run in the "flipped" orientation: the large streamed tensor
(K chunk / V chunk / weight block) is the stationary 128x128 lhsT so the
PE array is fully utilized and outputs come out pre-transposed - no PE
transposes anywhere.  Softmax skips max-subtraction (|scores| < ~0.5 by
construction); the sum S is taken with a ones-vector matmul and 1/S is
broadcast across partitions with a rank-1 fp32 matmul.

Per-core pipeline (b = 0,1 local batches):
  qT  = WqT64-blocks^T @ (queryT * SCALE/64)  + bq*SCALE      [128,8,2]
  tT  = Wk64-blocks^T @ blockdiag-masked qT                   [128,8,32]
  per b, per kpos-chunk kc (128 wide):
      scT[kc]  = Kp-tile^T @ tT(b)        (8 j-chunk accumulate)
      e[kc]    = exp(scT/64)              (ACT, bf16)
      S       += ones^T @ e[kc]           wv[jb] += V-tile^T @ e[kc]
  wvn[jb] = wv[jb] * bcast(1/S);  attn-pair[t2] = WvT-block^T @ wvn cols
  outT = WoT-blocks^T @ attn-pairs + bo2  (bo2 = bo + bv@Wo^T, host)
"""

import numpy as np
import ml_dtypes
from contextlib import ExitStack

import concourse.bass as bass
from concourse import bacc
import concourse.mybir as mybir
from concourse.tile import TileContext
from concourse.bass_utils import run_bass_kernel_spmd

B, SKV, D, H, HD = 16, 4096, 1024, 16, 64
NCORES = 8
BPC = B // NCORES  # 2 batches per core
SCALE = 1.0 / float(D) ** 0.5
C = 64.0  # fp8 pre-scale on Wq/Wk, undone in the exp()
NKC = SKV // 128  # 32 kpos chunks per batch
G = 8  # kpos chunks per K/V DMA group
NG = NKC // G

FP32 = mybir.dt.float32
BF16 = mybir.dt.bfloat16
FP8 = mybir.dt.float8e3
EXP = mybir.ActivationFunctionType.Exp

BF = np.dtype(ml_dtypes.bfloat16)
E3 = np.dtype(ml_dtypes.float8_e3m4)

_CACHE = {}


def build_nc():
    nc = bacc.Bacc("TRN2")

    # ---- kernel parameters (per core) ----
    qts = nc.declare_dram_parameter("qts", [128, 8, BPC], BF16, isOutput=False)
    Kp = nc.declare_dram_parameter("Kp", [BPC, NKC, 128, 8, 128], FP8, isOutput=False)
    Vn = nc.declare_dram_parameter("Vn", [BPC, SKV, D], FP8, isOutput=False)
    WqT64 = nc.declare_dram_parameter("WqT64", [D, D], FP8, isOutput=False)
    Wk64 = nc.declare_dram_parameter("Wk64", [D, D], FP8, isOutput=False)
    WvT = nc.declare_dram_parameter("WvT", [D, D], BF16, isOutput=False)
    WoT = nc.declare_dram_parameter("WoT", [D, D], BF16, isOutput=False)
    bqs2 = nc.declare_dram_parameter("bqs2", [128, 8, BPC], FP32, isOutput=False)
    bo22 = nc.declare_dram_parameter("bo22", [128, 8, BPC], FP32, isOutput=False)
    out_ext = nc.declare_dram_parameter("out", [BPC, D], FP32, isOutput=True)

    # [p, n, x] views (row r = n*128 + p)
    WqT_r = WqT64.rearrange("(n p) o -> p n o", p=128)
    Wk_r = Wk64.rearrange("(n p) o -> p n o", p=128)
    WvT_r = WvT.rearrange("(n p) o -> p n o", p=128)
    WoT_r = WoT.rearrange("(n p) o -> p n o", p=128)
    out_r = out_ext.rearrange("b (n p) -> p n b", p=128)

    with TileContext(nc) as tc, ExitStack() as ctx:
        consts = ctx.enter_context(tc.tile_pool(name="consts", bufs=1))
        wqk = ctx.enter_context(tc.tile_pool(name="wqk", bufs=2))
        kp = ctx.enter_context(tc.tile_pool(name="kp", bufs=2))
        vp = ctx.enter_context(tc.tile_pool(name="vp", bufs=2))
        elp = ctx.enter_context(tc.tile_pool(name="elp", bufs=2))
        wvnp = ctx.enter_context(tc.tile_pool(name="wvnp", bufs=16))
        small = ctx.enter_context(tc.tile_pool(name="small", bufs=4))
        ps_m = ctx.enter_context(tc.tile_pool(name="ps_m", bufs=2, space="PSUM"))
        ps_sc = ctx.enter_context(tc.tile_pool(name="ps_sc", bufs=2, space="PSUM"))
        ps_wv = ctx.enter_context(tc.tile_pool(name="ps_wv", bufs=2, space="PSUM"))
        ps_s = ctx.enter_context(tc.tile_pool(name="ps_s", bufs=1, space="PSUM"))

        # ---- small constants ----
        qts_sb = consts.tile([128, 8, BPC], BF16, tag="qts")
        nc.sync.dma_start(out=qts_sb, in_=qts[:, :, :])
        bqs2_sb = consts.tile([128, 8, BPC], FP32, tag="bqs2")
        nc.sync.dma_start(out=bqs2_sb, in_=bqs2[:, :, :])
        bo22_sb = consts.tile([128, 8, BPC], FP32, tag="bo22")
        nc.sync.dma_start(out=bo22_sb, in_=bo22[:, :, :])
        ones128 = consts.tile([128, 1], BF16, tag="ones128")
        nc.vector.memset(ones128, 1.0)
        ones1 = consts.tile([1, 128], FP32, tag="ones1")
        nc.vector.memset(ones1, 1.0)
        qmask = consts.tile([128, 8, 32], BF16, tag="qmask")
        nc.vector.memset(qmask, 0.0)

        # ---- qT = WqT-blocks^T @ qts + bqs (flipped: out [128 d, 8, b]) ----
        wq_sb = wqk.tile([128, 8, D], FP8, tag="w", name="wq_sb")
        nc.sync.dma_start(out=wq_sb, in_=WqT_r)
        wk_sb = wqk.tile([128, 8, D], FP8, tag="w", name="wk_sb")
        nc.sync.dma_start(out=wk_sb, in_=Wk_r)

        # db outer / ic inner: one open PSUM accumulation group per bank
        q_ps = ps_m.tile([128, 8, BPC], FP32, tag="m", name="q_ps")
        for db in range(8):
            for ic in range(8):
                nc.tensor.matmul(
                    q_ps[:, db, :],
                    wq_sb[:, ic, db * 128 : (db + 1) * 128],
                    qts_sb[:, ic, :],
                    start=(ic == 0),
                    stop=(ic == 7),
                )
        qt_sb = consts.tile([128, 8, BPC], BF16, tag="qt")
        for db in range(8):
            nc.vector.tensor_add(qt_sb[:, db, :], q_ps[:, db, :], bqs2_sb[:, db, :])

        # blockdiag mask: col 2h+b holds qT of head h (h = 2*ic + (p>=64))
        for ic in range(8):
            for b in range(BPC):
                nc.vector.tensor_copy(
                    qmask[0:64, ic, 4 * ic + b : 4 * ic + b + 1],
                    qt_sb[0:64, ic, b : b + 1],
                )
                nc.vector.tensor_copy(
                    qmask[64:128, ic, 4 * ic + 2 + b : 4 * ic + 3 + b],
                    qt_sb[64:128, ic, b : b + 1],
                )

        # ---- tT = Wk-blocks^T @ qmask (out [128 j, 8, 32(2h+b)]) ----
        t_ps = ps_m.tile([128, 8, 32], FP32, tag="m", name="t_ps")
        for jb in range(8):
            for ic in range(8):
                nc.tensor.matmul(
                    t_ps[:, jb, :],
                    wk_sb[:, ic, jb * 128 : (jb + 1) * 128],
                    qmask[:, ic, :],
                    start=(ic == 0),
                    stop=(ic == 7),
                )
        tT = [[None] * 8 for _ in range(BPC)]
        for jb in range(8):
            tv = t_ps[:, jb, :].rearrange("p (h b) -> p b h", b=BPC)
            for b in range(BPC):
                tt = consts.tile([128, 16], BF16, tag=f"tT{jb}_{b}", name=f"tT{jb}_{b}")
                nc.vector.tensor_copy(tt, tv[:, b, :])
                tT[b][jb] = tt

        # ---- per-batch attention ----
        attn_lhsT = [
            consts.tile([128, BPC], BF16, tag=f"al{t2}", name=f"al{t2}")
            for t2 in range(8)
        ]
        wvt_sb = None
        wot_sb = None
        for b in range(BPC):
            Kp_r = Kp[b].rearrange("kc p jc k -> p kc jc k")
            Vn_r = Vn[b].rearrange("(kc p) j -> p kc j", p=128)
            e_sb = elp.tile([128, NKC, 16], BF16, tag="e")
            S_ps = ps_s.tile([1, 16], FP32, tag="s")
            vres = vp.tile([128, NKC, D], FP8, tag="v", name="vres")
            for g in range(NG):
                kg = kp.tile([128, G, 8, 128], FP8, tag="k", name="kg")
                nc.sync.dma_start(out=kg, in_=Kp_r[:, g * G : (g + 1) * G, :, :])
                nc.sync.dma_start(
                    out=vres[:, g * G : (g + 1) * G, :],
                    in_=Vn_r[:, g * G : (g + 1) * G, :],
                )
                for c in range(G):
                    kc = g * G + c
                    sc = ps_sc.tile([128, 16], FP32, tag="sc")
                    for jc in range(8):
                        nc.tensor.matmul(
                            sc,
                            kg[:, c, jc, :],
                            tT[b][jc],
                            start=(jc == 0),
                            stop=(jc == 7),
                        )
                    nc.scalar.activation(
                        out=e_sb[:, kc, :], in_=sc, func=EXP, bias=0.0, scale=1.0 / C
                    )
                    nc.tensor.matmul(
                        S_ps,
                        ones128,
                        e_sb[:, kc, :],
                        start=(kc == 0),
                        stop=(kc == NKC - 1),
                    )
            # wv pass: V is resident, one accumulation group (bank) at a time
            wv_ps = ps_wv.tile([128, 8, 16], FP32, tag="wv")
            for jb in range(8):
                for kc in range(NKC):
                    nc.tensor.matmul(
                        wv_ps[:, jb, :],
                        vres[:, kc, jb * 128 : (jb + 1) * 128],
                        e_sb[:, kc, :],
                        start=(kc == 0),
                        stop=(kc == NKC - 1),
                    )
            # WvT/WoT loads go in the DMA queue after batch 0's K/V stream
            if b == 0:
                wvt_sb = consts.tile([128, 8, D], BF16, tag="wvt")
                nc.sync.dma_start(out=wvt_sb, in_=WvT_r)
                wot_sb = consts.tile([128, 8, D], BF16, tag="wot")
                nc.sync.dma_start(out=wot_sb, in_=WoT_r)

            # 1/S broadcast across partitions (rank-1 fp32 matmul)
            rS_sb = small.tile([1, 16], FP32, tag="rs", name="rS_sb")
            nc.vector.reciprocal(rS_sb, S_ps)
            Sb_ps = ps_m.tile([128, 16], FP32, tag="m", name="Sb_ps")
            nc.tensor.matmul(Sb_ps, ones1, rS_sb, start=True, stop=True)
            Sb_sb = small.tile([128, 16], FP32, tag="sb", name="Sb_sb")
            nc.vector.tensor_copy(Sb_sb, Sb_ps)

            wvn = []
            for jb in range(8):
                w1 = wvnp.tile([128, 16], BF16, tag="wvn", name="w1")
                nc.vector.tensor_mul(w1, wv_ps[:, jb, :], Sb_sb)
                wvn.append(w1)

            # attn pairs: at2 [128(hd pair), 2(head)] per t2; diagonal extract
            for t2 in range(8):
                at2 = ps_m.tile([128, BPC], FP32, tag="m", name="at2")
                for jc in range(8):
                    nc.tensor.matmul(
                        at2,
                        wvt_sb[:, jc, t2 * 128 : (t2 + 1) * 128],
                        wvn[jc][:, 2 * t2 : 2 * t2 + 2],
                        start=(jc == 0),
                        stop=(jc == 7),
                    )
                nc.vector.tensor_copy(
                    attn_lhsT[t2][0:64, b : b + 1], at2[0:64, 0:1]
                )
                nc.vector.tensor_copy(
                    attn_lhsT[t2][64:128, b : b + 1], at2[64:128, 1:2]
                )

        # ---- out = WoT-blocks^T @ attn + bo2 (outT [128 o, 8, b]) ----
        o_ps = ps_m.tile([128, 8, BPC], FP32, tag="m", name="o_ps")
        for ob in range(8):
            for t2 in range(8):
                nc.tensor.matmul(
                    o_ps[:, ob, :],
                    wot_sb[:, t2, ob * 128 : (ob + 1) * 128],
                    attn_lhsT[t2],
                    start=(t2 == 0),
                    stop=(t2 == 7),
                )
        out_sb = consts.tile([128, 8, BPC], FP32, tag="out")
        for ob in range(8):
            nc.vector.tensor_add(out_sb[:, ob, :], o_ps[:, ob, :], bo22_sb[:, ob, :])
        for b in range(BPC):
            nc.sync.dma_start(out=out_r[:, :, b], in_=out_sb[:, :, b])

    if not nc.is_finalized():
        nc.finalize()
    return nc


def _prep_in_maps(inputs):
    query = np.asarray(inputs["query"], np.float32)
    key = np.asarray(inputs["key"], np.float32)
    value = np.asarray(inputs["value"], np.float32)
    Wq = np.asarray(inputs["Wq"], np.float32)
    bq = np.asarray(inputs["bq"], np.float32)
    Wk = np.asarray(inputs["Wk"], np.float32)
    Wv = np.asarray(inputs["Wv"], np.float32)
    Wo = np.asarray(inputs["Wo"], np.float32)
    bv = np.asarray(inputs["bv"], np.float32)
    bo = np.asarray(inputs["bo"], np.float32)

    bo2 = bo + bv @ Wo.T  # Sum of softmax weights = 1 folds bv through Wo
    shared = {
        "WqT64": np.ascontiguousarray(Wq.T * C).astype(E3),
        "Wk64": np.ascontiguousarray(Wk * C).astype(E3),
        "WvT": np.ascontiguousarray(Wv.T).astype(BF),
        "WoT": np.ascontiguousarray(Wo.T).astype(BF),
        "bqs2": np.ascontiguousarray(
            np.repeat((bq * SCALE).reshape(8, 128).T[:, :, None], BPC, axis=2)
        ),
        "bo22": np.ascontiguousarray(
            np.repeat(bo2.reshape(8, 128).T[:, :, None], BPC, axis=2)
        ),
    }
    in_maps = []
    for c in range(NCORES):
        c0 = c * BPC
        qt = (query[c0 : c0 + BPC, 0, :].T * (SCALE / C)).astype(BF)  # [D, BPC]
        in_maps.append(
            {
                "qts": np.ascontiguousarray(qt.reshape(8, 128, BPC).transpose(1, 0, 2)),
                "Kp": np.ascontiguousarray(
                    key[c0 : c0 + BPC]
                    .astype(E3)
                    .reshape(BPC, NKC, 128, 8, 128)
                    .transpose(0, 1, 4, 3, 2)
                ),
                "Vn": np.ascontiguousarray(value[c0 : c0 + BPC].astype(E3)),
                **shared,
            }
        )
    return in_maps


def kernel(**inputs):
    if "nc" not in _CACHE:
        _CACHE["nc"] = build_nc()
    nc = _CACHE["nc"]
    in_maps = _prep_in_maps(inputs)
    res = run_bass_kernel_spmd(nc, in_maps, list(range(NCORES)))
    return np.concatenate([res.results[i]["out"] for i in range(NCORES)], axis=0)


if __name__ == "__main__":
    nc = build_nc()
    print("built ok")


# revision 15
# speedup vs baseline: 2.3513x; 1.0117x over previous
"""Trainium2 Bass kernel for nn_CrossAttention (B=16, SQ=1, SKV=4096, D=1024, H=16).

Strategy
--------
Data-parallel over batch: each of the 8 cores owns 2 batch elements.

Because SQ == 1 the K/V projections fold away:

  scores[h,kpos] = t[h,:] . key[kpos,:],   t = SCALE * blockdiag(qh) @ Wk
    (bk is constant along kpos -> cancels in softmax)
  wv[h,:]   = e[h,:] @ value            (raw value, project after)
  attn[h,:] = (wv[h,:]/S) @ Wv_h^T      (+ bv folds into bo on host: Sum w = 1)

This drops compute ~64x vs the naive form and makes the kernel DMA-bound
on streaming K and V once.  K/V (and Wq/Wk, x64-scaled to dodge e3m4
subnormals) are cast to fp8 e3m4 on the host, halving DMA vs bf16;
Wv/Wo stay bf16 (fp8 there is too lossy).  Measured end-to-end error
vs the fp32 reference: ~4e-3 Frobenius-relative.

All matmuls run in the "flipped" orientation: the large streamed tensor
(K chunk / V chunk / weight block) is the stationary 128x128 lhsT so the
PE array is fully utilized and outputs come out pre-transposed - no PE
transposes anywhere.  Softmax skips max-subtraction (|scores| < ~0.5 by
construction); the sum S is taken with a ones-vector matmul and 1/S is
broadcast across partitions with a rank-1 fp32 matmul.

Per-core pipeline (b = 0,1 local batches):
  qT  = WqT64-blocks^T @ (queryT * SCALE/64)  + bq*SCALE      [128,8,2]
  tT  = Wk64-blocks^T @ blockdiag-masked qT                   [128,8,32]
  per b, per kpos-chunk kc (128 wide):
      scT[kc]  = Kp-tile^T @ tT(b)        (8 j-chunk accumulate)
      e[kc]    = exp(scT/64)              (ACT, bf16)
      S       += ones^T @ e[kc]           wv[jb] += V-tile^T @ e[kc]
  wvn[jb] = wv[jb] * bcast(1/S);  attn-pair[t2] = WvT-block^T @ wvn cols
  outT = WoT-blocks^T @ attn-pairs + bo2  (bo2 = bo + bv@Wo^T, host)
"""

import numpy as np
import ml_dtypes
from contextlib import ExitStack

import concourse.bass as bass
from concourse import bacc
import concourse.mybir as mybir
from concourse.tile import TileContext
from concourse.bass_utils import run_bass_kernel_spmd

B, SKV, D, H, HD = 16, 4096, 1024, 16, 64
NCORES = 8
BPC = B // NCORES  # 2 batches per core
SCALE = 1.0 / float(D) ** 0.5
C = 64.0  # fp8 pre-scale on Wq/Wk, undone in the exp()
NKC = SKV // 128  # 32 kpos chunks per batch
G = 16  # kpos chunks per K/V DMA group
NG = NKC // G

FP32 = mybir.dt.float32
BF16 = mybir.dt.bfloat16
FP8 = mybir.dt.float8e3
EXP = mybir.ActivationFunctionType.Exp

BF = np.dtype(ml_dtypes.bfloat16)
E3 = np.dtype(ml_dtypes.float8_e3m4)

_CACHE = {}


def build_nc():
    nc = bacc.Bacc("TRN2")

    # ---- kernel parameters (per core) ----
    # smalls packs qts (cols 0:2), bq*SCALE (2:4), bo2 (4:6) into one DMA
    smalls = nc.declare_dram_parameter("smalls", [128, 8, 6], FP32, isOutput=False)
    Kp = nc.declare_dram_parameter("Kp", [BPC, NKC, 128, 8, 128], FP8, isOutput=False)
    Vn = nc.declare_dram_parameter("Vn", [BPC, SKV, D], FP8, isOutput=False)
    WqT64 = nc.declare_dram_parameter("WqT64", [D, D], FP8, isOutput=False)
    Wk64 = nc.declare_dram_parameter("Wk64", [D, D], FP8, isOutput=False)
    WvT = nc.declare_dram_parameter("WvT", [D, D], BF16, isOutput=False)
    WoT = nc.declare_dram_parameter("WoT", [D, D], BF16, isOutput=False)
    out_ext = nc.declare_dram_parameter("out", [128, 8, BPC], FP32, isOutput=True)

    # [p, n, x] views (row r = n*128 + p)
    WqT_r = WqT64.rearrange("(n p) o -> p n o", p=128)
    Wk_r = Wk64.rearrange("(n p) o -> p n o", p=128)
    WvT_r = WvT.rearrange("(n p) o -> p n o", p=128)
    WoT_r = WoT.rearrange("(n p) o -> p n o", p=128)

    with TileContext(nc) as tc, ExitStack() as ctx:
        consts = ctx.enter_context(tc.tile_pool(name="consts", bufs=1))
        wqk = ctx.enter_context(tc.tile_pool(name="wqk", bufs=2))
        kp = ctx.enter_context(tc.tile_pool(name="kp", bufs=2))
        vp = ctx.enter_context(tc.tile_pool(name="vp", bufs=2))
        elp = ctx.enter_context(tc.tile_pool(name="elp", bufs=2))
        wvnp = ctx.enter_context(tc.tile_pool(name="wvnp", bufs=16))
        small = ctx.enter_context(tc.tile_pool(name="small", bufs=4))
        ps_m = ctx.enter_context(tc.tile_pool(name="ps_m", bufs=2, space="PSUM"))
        ps_sc = ctx.enter_context(tc.tile_pool(name="ps_sc", bufs=2, space="PSUM"))
        ps_wv = ctx.enter_context(tc.tile_pool(name="ps_wv", bufs=2, space="PSUM"))
        ps_s = ctx.enter_context(tc.tile_pool(name="ps_s", bufs=1, space="PSUM"))

        # ---- weight + small-constant DMAs (queue order: wq, smalls, wk) ----
        wq_sb = wqk.tile([128, 8, D], FP8, tag="w", name="wq_sb")
        nc.sync.dma_start(out=wq_sb, in_=WqT_r)
        smalls_sb = consts.tile([128, 8, 6], FP32, tag="smalls")
        nc.sync.dma_start(out=smalls_sb, in_=smalls[:, :, :])
        wk_sb = wqk.tile([128, 8, D], FP8, tag="w", name="wk_sb")
        nc.sync.dma_start(out=wk_sb, in_=Wk_r)

        bqs2_sb = smalls_sb[:, :, 2:4]
        bo22_sb = smalls_sb[:, :, 4:6]
        qts_sb = consts.tile([128, 8, BPC], BF16, tag="qts")
        nc.vector.tensor_copy(qts_sb, smalls_sb[:, :, 0:2])
        ones128 = consts.tile([128, 1], BF16, tag="ones128")
        nc.vector.memset(ones128, 1.0)
        ones1 = consts.tile([1, 128], FP32, tag="ones1")
        nc.vector.memset(ones1, 1.0)
        qmask = consts.tile([128, 8, 32], BF16, tag="qmask")
        nc.vector.memset(qmask, 0.0)

        # db outer / ic inner: one open PSUM accumulation group per bank
        q_ps = ps_m.tile([128, 8, BPC], FP32, tag="m", name="q_ps")
        for db in range(8):
            for ic in range(8):
                nc.tensor.matmul(
                    q_ps[:, db, :],
                    wq_sb[:, ic, db * 128 : (db + 1) * 128],
                    qts_sb[:, ic, :],
                    start=(ic == 0),
                    stop=(ic == 7),
                )
        qt_sb = consts.tile([128, 8, BPC], BF16, tag="qt")
        for db in range(8):
            nc.vector.tensor_add(qt_sb[:, db, :], q_ps[:, db, :], bqs2_sb[:, db, :])

        # blockdiag mask: col 2h+b holds qT of head h (h = 2*ic + (p>=64))
        for ic in range(8):
            for b in range(BPC):
                nc.vector.tensor_copy(
                    qmask[0:64, ic, 4 * ic + b : 4 * ic + b + 1],
                    qt_sb[0:64, ic, b : b + 1],
                )
                nc.vector.tensor_copy(
                    qmask[64:128, ic, 4 * ic + 2 + b : 4 * ic + 3 + b],
                    qt_sb[64:128, ic, b : b + 1],
                )

        # ---- tT = Wk-blocks^T @ qmask (out [128 j, 8, 32(2h+b)]) ----
        t_ps = ps_m.tile([128, 8, 32], FP32, tag="m", name="t_ps")
        for jb in range(8):
            for ic in range(8):
                nc.tensor.matmul(
                    t_ps[:, jb, :],
                    wk_sb[:, ic, jb * 128 : (jb + 1) * 128],
                    qmask[:, ic, :],
                    start=(ic == 0),
                    stop=(ic == 7),
                )
        tT = [[None] * 8 for _ in range(BPC)]
        for jb in range(8):
            tv = t_ps[:, jb, :].rearrange("p (h b) -> p b h", b=BPC)
            for b in range(BPC):
                tt = consts.tile([128, 16], BF16, tag=f"tT{jb}_{b}", name=f"tT{jb}_{b}")
                nc.vector.tensor_copy(tt, tv[:, b, :])
                tT[b][jb] = tt

        # ---- per-batch attention ----
        attn_lhsT = [
            consts.tile([128, BPC], BF16, tag=f"al{t2}", name=f"al{t2}")
            for t2 in range(8)
        ]
        wvt_sb = None
        wot_sb = None
        for b in range(BPC):
            Kp_r = Kp[b].rearrange("kc p jc k -> p kc jc k")
            Vn_r = Vn[b].rearrange("(kc p) j -> p kc j", p=128)
            e_sb = elp.tile([128, NKC, 16], BF16, tag="e")
            S_ps = ps_s.tile([1, 16], FP32, tag="s")
            vres = vp.tile([128, NKC, D], FP8, tag="v", name="vres")
            for g in range(NG):
                kg = kp.tile([128, G, 8, 128], FP8, tag="k", name="kg")
                nc.sync.dma_start(out=kg, in_=Kp_r[:, g * G : (g + 1) * G, :, :])
                nc.sync.dma_start(
                    out=vres[:, g * G : (g + 1) * G, :],
                    in_=Vn_r[:, g * G : (g + 1) * G, :],
                )
                # WvT/WoT go in the DMA queue right behind batch 0's stream
                if b == 0 and g == NG - 1:
                    wvt_sb = consts.tile([128, 8, D], BF16, tag="wvt")
                    nc.sync.dma_start(out=wvt_sb, in_=WvT_r)
                    wot_sb = consts.tile([128, 8, D], BF16, tag="wot")
                    nc.sync.dma_start(out=wot_sb, in_=WoT_r)
                for c in range(G):
                    kc = g * G + c
                    sc = ps_sc.tile([128, 16], FP32, tag="sc")
                    for jc in range(8):
                        nc.tensor.matmul(
                            sc,
                            kg[:, c, jc, :],
                            tT[b][jc],
                            start=(jc == 0),
                            stop=(jc == 7),
                        )
                    nc.scalar.activation(
                        out=e_sb[:, kc, :], in_=sc, func=EXP, bias=0.0, scale=1.0 / C
                    )
            # S after the kc loop so it never head-of-line-blocks the PE
            for kc in range(NKC):
                nc.tensor.matmul(
                    S_ps,
                    ones128,
                    e_sb[:, kc, :],
                    start=(kc == 0),
                    stop=(kc == NKC - 1),
                )
            # wv pass: V is resident, one accumulation group (bank) at a time
            wv_ps = ps_wv.tile([128, 8, 16], FP32, tag="wv")
            for jb in range(8):
                for kc in range(NKC):
                    nc.tensor.matmul(
                        wv_ps[:, jb, :],
                        vres[:, kc, jb * 128 : (jb + 1) * 128],
                        e_sb[:, kc, :],
                        start=(kc == 0),
                        stop=(kc == NKC - 1),
                    )

            # 1/S broadcast across partitions (rank-1 fp32 matmul)
            rS_sb = small.tile([1, 16], FP32, tag="rs", name="rS_sb")
            nc.vector.reciprocal(rS_sb, S_ps)
            Sb_ps = ps_m.tile([128, 16], FP32, tag="m", name="Sb_ps")
            nc.tensor.matmul(Sb_ps, ones1, rS_sb, start=True, stop=True)
            Sb_sb = small.tile([128, 16], FP32, tag="sb", name="Sb_sb")
            nc.vector.tensor_copy(Sb_sb, Sb_ps)

            wvn = []
            for jb in range(8):
                w1 = wvnp.tile([128, 16], BF16, tag="wvn", name="w1")
                nc.vector.tensor_mul(w1, wv_ps[:, jb, :], Sb_sb)
                wvn.append(w1)

            # attn pairs: at2 [128(hd pair), 2(head)] per t2; diagonal extract
            for t2 in range(8):
                at2 = ps_m.tile([128, BPC], FP32, tag="m", name="at2")
                for jc in range(8):
                    nc.tensor.matmul(
                        at2,
                        wvt_sb[:, jc, t2 * 128 : (t2 + 1) * 128],
                        wvn[jc][:, 2 * t2 : 2 * t2 + 2],
                        start=(jc == 0),
                        stop=(jc == 7),
                    )
                nc.scalar.copy(attn_lhsT[t2][0:64, b : b + 1], at2[0:64, 0:1])
                nc.scalar.copy(attn_lhsT[t2][64:128, b : b + 1], at2[64:128, 1:2])

        # ---- out = WoT-blocks^T @ attn + bo2 (outT [128 o, 8, b]) ----
        o_ps = ps_m.tile([128, 8, BPC], FP32, tag="m", name="o_ps")
        for ob in range(8):
            for t2 in range(8):
                nc.tensor.matmul(
                    o_ps[:, ob, :],
                    wot_sb[:, t2, ob * 128 : (ob + 1) * 128],
                    attn_lhsT[t2],
                    start=(t2 == 0),
                    stop=(t2 == 7),
                )
        out_sb = consts.tile([128, 8, BPC], FP32, tag="out")
        for ob in range(8):
            nc.vector.tensor_add(out_sb[:, ob, :], o_ps[:, ob, :], bo22_sb[:, ob, :])
        nc.sync.dma_start(out=out_ext[:, :, :], in_=out_sb)

    if not nc.is_finalized():
        nc.finalize()
    return nc


def _prep_in_maps(inputs):
    query = np.asarray(inputs["query"], np.float32)
    key = np.asarray(inputs["key"], np.float32)
    value = np.asarray(inputs["value"], np.float32)
    Wq = np.asarray(inputs["Wq"], np.float32)
    bq = np.asarray(inputs["bq"], np.float32)
    Wk = np.asarray(inputs["Wk"], np.float32)
    Wv = np.asarray(inputs["Wv"], np.float32)
    Wo = np.asarray(inputs["Wo"], np.float32)
    bv = np.asarray(inputs["bv"], np.float32)
    bo = np.asarray(inputs["bo"], np.float32)

    bo2 = bo + bv @ Wo.T  # Sum of softmax weights = 1 folds bv through Wo
    shared = {
        "WqT64": np.ascontiguousarray(Wq.T * C).astype(E3),
        "Wk64": np.ascontiguousarray(Wk * C).astype(E3),
        "WvT": np.ascontiguousarray(Wv.T).astype(BF),
        "WoT": np.ascontiguousarray(Wo.T).astype(BF),
    }
    bqs_pack = (bq * SCALE).reshape(8, 128).T[:, :, None]  # [128, 8, 1]
    bo2_pack = bo2.reshape(8, 128).T[:, :, None]
    in_maps = []
    for c in range(NCORES):
        c0 = c * BPC
        qt = query[c0 : c0 + BPC, 0, :].T * (SCALE / C)  # [D, BPC] fp32
        qt_pack = qt.reshape(8, 128, BPC).transpose(1, 0, 2)  # [128, 8, 2]
        smalls = np.concatenate(
            [
                qt_pack,
                np.broadcast_to(bqs_pack, (128, 8, BPC)),
                np.broadcast_to(bo2_pack, (128, 8, BPC)),
            ],
            axis=2,
        )
        in_maps.append(
            {
                "smalls": np.ascontiguousarray(smalls, np.float32),
                "Kp": np.ascontiguousarray(
                    key[c0 : c0 + BPC]
                    .astype(E3)
                    .reshape(BPC, NKC, 128, 8, 128)
                    .transpose(0, 1, 4, 3, 2)
                ),
                "Vn": np.ascontiguousarray(value[c0 : c0 + BPC].astype(E3)),
                **shared,
            }
        )
    return in_maps


def kernel(**inputs):
    if "nc" not in _CACHE:
        _CACHE["nc"] = build_nc()
    nc = _CACHE["nc"]
    in_maps = _prep_in_maps(inputs)
    res = run_bass_kernel_spmd(nc, in_maps, list(range(NCORES)))
    # device gives outT [128 p, 8 n, BPC b]; full[b, n*128+p] = outT[p, n, b]
    return np.concatenate(
        [
            res.results[i]["out"].transpose(2, 1, 0).reshape(BPC, D)
            for i in range(NCORES)
        ],
        axis=0,
    )


if __name__ == "__main__":
    nc = build_nc()
    print("built ok")


# revision 22
# speedup vs baseline: 3.0056x; 1.2783x over previous
"""Trainium2 Bass kernel for nn_CrossAttention (B=16, SQ=1, SKV=4096, D=1024, H=16).

Strategy
--------
Data-parallel over batch: each of the 8 cores owns 2 batch elements.

Because SQ == 1 the K/V projections fold away:

  scores[h,kpos] = t[h,:] . key[kpos,:],   t = SCALE * blockdiag(qh) @ Wk
    (bk is constant along kpos -> cancels in softmax)
  wv[h,:]   = e[h,:] @ value            (raw value, project after)
  attn[h,:] = (wv[h,:]/S) @ Wv_h^T      (+ bv folds into bo on host: Sum w = 1)

This drops compute ~64x vs the naive form and makes the kernel DMA-bound
on streaming K and V once.  K/V and all four weights are cast to fp8
e3m4 on the host (weights x64-scaled to dodge e3m4 subnormals; the two
1/64 factors for Wv/Wo fold into the 1/S broadcast constant), halving
DMA vs bf16.  K-chunk DMAs ride the SP HWDGE queue and V-chunk/weight
DMAs the ACT queue so per-DMA setup latencies overlap.  Measured
end-to-end error vs the fp32 reference: ~6e-3 Frobenius-relative.

All matmuls run in the "flipped" orientation: the large streamed tensor
(K chunk / V chunk / weight block) is the stationary 128x128 lhsT so the
PE array is fully utilized and outputs come out pre-transposed - no PE
transposes anywhere.  Softmax skips max-subtraction (|scores| < ~0.5 by
construction); the sum S is taken with a ones-vector matmul and 1/S is
broadcast across partitions with a rank-1 fp32 matmul.

Per-core pipeline (b = 0,1 local batches):
  qT  = WqT64-blocks^T @ (queryT * SCALE/64)  + bq*SCALE      [128,8,2]
  tT  = Wk64-blocks^T @ blockdiag-masked qT                   [128,8,32]
  per b, per kpos-chunk kc (128 wide):
      scT[kc]  = Kp-tile^T @ tT(b)        (8 j-chunk accumulate)
      e[kc]    = exp(scT/64)              (ACT, bf16)
      S       += ones^T @ e[kc]           wv[jb] += V-tile^T @ e[kc]
  wvn[jb] = wv[jb] * bcast(1/S);  attn-pair[t2] = WvT-block^T @ wvn cols
  outT = WoT-blocks^T @ attn-pairs + bo2  (bo2 = bo + bv@Wo^T, host)
"""

import numpy as np
import ml_dtypes
from contextlib import ExitStack

import concourse.bass as bass
from concourse import bacc
import concourse.mybir as mybir
from concourse.tile import TileContext
from concourse.bass_utils import run_bass_kernel_spmd

B, SKV, D, H, HD = 16, 4096, 1024, 16, 64
NCORES = 8
BPC = B // NCORES  # 2 batches per core
SCALE = 1.0 / float(D) ** 0.5
C = 64.0  # fp8 pre-scale on Wq/Wk, undone in the exp()
NKC = SKV // 128  # 32 kpos chunks per batch
G = 16  # kpos chunks per K/V DMA group
NG = NKC // G

FP32 = mybir.dt.float32
BF16 = mybir.dt.bfloat16
FP8 = mybir.dt.float8e3
EXP = mybir.ActivationFunctionType.Exp

BF = np.dtype(ml_dtypes.bfloat16)
E3 = np.dtype(ml_dtypes.float8_e3m4)

_CACHE = {}


def build_nc():
    nc = bacc.Bacc("TRN2")

    # ---- kernel parameters (per core) ----
    # smalls packs qts (cols 0:2), bq*SCALE (2:4), bo2 (4:6) into one DMA
    smalls = nc.declare_dram_parameter("smalls", [128, 8, 6], FP32, isOutput=False)
    Kp = nc.declare_dram_parameter("Kp", [BPC, NKC, 128, 8, 128], FP8, isOutput=False)
    Vn = nc.declare_dram_parameter("Vn", [BPC, SKV, D], FP8, isOutput=False)
    WqT64 = nc.declare_dram_parameter("WqT64", [D, D], FP8, isOutput=False)
    Wk64 = nc.declare_dram_parameter("Wk64", [D, D], FP8, isOutput=False)
    WvT64 = nc.declare_dram_parameter("WvT64", [D, D], FP8, isOutput=False)
    WoT64 = nc.declare_dram_parameter("WoT64", [D, D], FP8, isOutput=False)
    out_ext = nc.declare_dram_parameter("out", [128, 8, BPC], FP32, isOutput=True)

    # [p, n, x] views (row r = n*128 + p)
    WqT_r = WqT64.rearrange("(n p) o -> p n o", p=128)
    Wk_r = Wk64.rearrange("(n p) o -> p n o", p=128)
    WvT_r = WvT64.rearrange("(n p) o -> p n o", p=128)
    WoT_r = WoT64.rearrange("(n p) o -> p n o", p=128)

    with TileContext(nc) as tc, ExitStack() as ctx:
        consts = ctx.enter_context(tc.tile_pool(name="consts", bufs=1))
        wqk = ctx.enter_context(tc.tile_pool(name="wqk", bufs=2))
        kp = ctx.enter_context(tc.tile_pool(name="kp", bufs=2))
        vp = ctx.enter_context(tc.tile_pool(name="vp", bufs=2))
        elp = ctx.enter_context(tc.tile_pool(name="elp", bufs=2))
        wvnp = ctx.enter_context(tc.tile_pool(name="wvnp", bufs=16))
        small = ctx.enter_context(tc.tile_pool(name="small", bufs=4))
        ps_m = ctx.enter_context(tc.tile_pool(name="ps_m", bufs=2, space="PSUM"))
        ps_sc = ctx.enter_context(tc.tile_pool(name="ps_sc", bufs=2, space="PSUM"))
        ps_wv = ctx.enter_context(tc.tile_pool(name="ps_wv", bufs=2, space="PSUM"))
        ps_s = ctx.enter_context(tc.tile_pool(name="ps_s", bufs=1, space="PSUM"))

        # ---- weight + small-constant DMAs, alternating the SP and ACT
        # HWDGE queues so per-DMA setup latencies overlap ----
        wq_sb = wqk.tile([128, 8, D], FP8, tag="w", name="wq_sb")
        nc.sync.dma_start(out=wq_sb, in_=WqT_r)
        smalls_sb = consts.tile([128, 8, 6], FP32, tag="smalls")
        nc.scalar.dma_start(out=smalls_sb, in_=smalls[:, :, :])
        wk_sb = wqk.tile([128, 8, D], FP8, tag="w", name="wk_sb")
        nc.scalar.dma_start(out=wk_sb, in_=Wk_r)
        wvt_sb = consts.tile([128, 8, D], FP8, tag="wvt")
        nc.sync.dma_start(out=wvt_sb, in_=WvT_r)
        wot_sb = consts.tile([128, 8, D], FP8, tag="wot")
        nc.scalar.dma_start(out=wot_sb, in_=WoT_r)

        bqs2_sb = smalls_sb[:, :, 2:4]
        bo22_sb = smalls_sb[:, :, 4:6]
        qts_sb = consts.tile([128, 8, BPC], BF16, tag="qts")
        nc.vector.tensor_copy(qts_sb, smalls_sb[:, :, 0:2])
        ones128 = consts.tile([128, 1], BF16, tag="ones128")
        nc.vector.memset(ones128, 1.0)
        # 1/(C*C) undoes the x64 pre-scale on both WvT64 and WoT64
        ones1 = consts.tile([1, 128], FP32, tag="ones1")
        nc.vector.memset(ones1, 1.0 / (C * C))
        qmask = consts.tile([128, 8, 32], BF16, tag="qmask")
        nc.vector.memset(qmask, 0.0)

        # db outer / ic inner: one open PSUM accumulation group per bank
        q_ps = ps_m.tile([128, 8, BPC], FP32, tag="m", name="q_ps")
        for db in range(8):
            for ic in range(8):
                nc.tensor.matmul(
                    q_ps[:, db, :],
                    wq_sb[:, ic, db * 128 : (db + 1) * 128],
                    qts_sb[:, ic, :],
                    start=(ic == 0),
                    stop=(ic == 7),
                )
        qt_sb = consts.tile([128, 8, BPC], BF16, tag="qt")
        for db in range(8):
            nc.vector.tensor_add(qt_sb[:, db, :], q_ps[:, db, :], bqs2_sb[:, db, :])

        # blockdiag mask: col 2h+b holds qT of head h (h = 2*ic + (p>=64))
        for ic in range(8):
            for b in range(BPC):
                nc.vector.tensor_copy(
                    qmask[0:64, ic, 4 * ic + b : 4 * ic + b + 1],
                    qt_sb[0:64, ic, b : b + 1],
                )
                nc.vector.tensor_copy(
                    qmask[64:128, ic, 4 * ic + 2 + b : 4 * ic + 3 + b],
                    qt_sb[64:128, ic, b : b + 1],
                )

        # ---- tT = Wk-blocks^T @ qmask (out [128 j, 8, 32(2h+b)]) ----
        t_ps = ps_m.tile([128, 8, 32], FP32, tag="m", name="t_ps")
        for jb in range(8):
            for ic in range(8):
                nc.tensor.matmul(
                    t_ps[:, jb, :],
                    wk_sb[:, ic, jb * 128 : (jb + 1) * 128],
                    qmask[:, ic, :],
                    start=(ic == 0),
                    stop=(ic == 7),
                )
        tT = [[None] * 8 for _ in range(BPC)]
        for jb in range(8):
            tv = t_ps[:, jb, :].rearrange("p (h b) -> p b h", b=BPC)
            for b in range(BPC):
                tt = consts.tile([128, 16], BF16, tag=f"tT{jb}_{b}", name=f"tT{jb}_{b}")
                nc.vector.tensor_copy(tt, tv[:, b, :])
                tT[b][jb] = tt

        # ---- per-batch attention ----
        attn_lhsT = [
            consts.tile([128, BPC], BF16, tag=f"al{t2}", name=f"al{t2}")
            for t2 in range(8)
        ]
        for b in range(BPC):
            Kp_r = Kp[b].rearrange("kc p jc k -> p kc jc k")
            Vn_r = Vn[b].rearrange("(kc p) j -> p kc j", p=128)
            e_sb = elp.tile([128, NKC, 16], BF16, tag="e")
            S_ps = ps_s.tile([1, 16], FP32, tag="s")
            vres = vp.tile([128, NKC, D], FP8, tag="v", name="vres")
            for g in range(NG):
                kg = kp.tile([128, G, 8, 128], FP8, tag="k", name="kg")
                nc.sync.dma_start(out=kg, in_=Kp_r[:, g * G : (g + 1) * G, :, :])
                nc.scalar.dma_start(
                    out=vres[:, g * G : (g + 1) * G, :],
                    in_=Vn_r[:, g * G : (g + 1) * G, :],
                )
                for c in range(G):
                    kc = g * G + c
                    sc = ps_sc.tile([128, 16], FP32, tag="sc")
                    for jc in range(8):
                        nc.tensor.matmul(
                            sc,
                            kg[:, c, jc, :],
                            tT[b][jc],
                            start=(jc == 0),
                            stop=(jc == 7),
                        )
                    nc.scalar.activation(
                        out=e_sb[:, kc, :], in_=sc, func=EXP, bias=0.0, scale=1.0 / C
                    )
            # S after the kc loop so it never head-of-line-blocks the PE
            for kc in range(NKC):
                nc.tensor.matmul(
                    S_ps,
                    ones128,
                    e_sb[:, kc, :],
                    start=(kc == 0),
                    stop=(kc == NKC - 1),
                )
            # wv pass: V is resident, one accumulation group (bank) at a time
            wv_ps = ps_wv.tile([128, 8, 16], FP32, tag="wv")
            for jb in range(8):
                for kc in range(NKC):
                    nc.tensor.matmul(
                        wv_ps[:, jb, :],
                        vres[:, kc, jb * 128 : (jb + 1) * 128],
                        e_sb[:, kc, :],
                        start=(kc == 0),
                        stop=(kc == NKC - 1),
                    )

            # 1/S broadcast across partitions (rank-1 fp32 matmul)
            rS_sb = small.tile([1, 16], FP32, tag="rs", name="rS_sb")
            nc.vector.reciprocal(rS_sb, S_ps)
            Sb_ps = ps_m.tile([128, 16], FP32, tag="m", name="Sb_ps")
            nc.tensor.matmul(Sb_ps, ones1, rS_sb, start=True, stop=True)
            Sb_sb = small.tile([128, 16], FP32, tag="sb", name="Sb_sb")
            nc.vector.tensor_copy(Sb_sb, Sb_ps)

            wvn = []
            for jb in range(8):
                w1 = wvnp.tile([128, 16], BF16, tag="wvn", name="w1")
                nc.vector.tensor_mul(w1, wv_ps[:, jb, :], Sb_sb)
                wvn.append(w1)

            # attn pairs: at2 [128(hd pair), 2(head)] per t2; diagonal extract
            for t2 in range(8):
                at2 = ps_m.tile([128, BPC], FP32, tag="m", name="at2")
                for jc in range(8):
                    nc.tensor.matmul(
                        at2,
                        wvt_sb[:, jc, t2 * 128 : (t2 + 1) * 128],
                        wvn[jc][:, 2 * t2 : 2 * t2 + 2],
                        start=(jc == 0),
                        stop=(jc == 7),
                    )
                nc.scalar.copy(attn_lhsT[t2][0:64, b : b + 1], at2[0:64, 0:1])
                nc.scalar.copy(attn_lhsT[t2][64:128, b : b + 1], at2[64:128, 1:2])

        # ---- out = WoT-blocks^T @ attn + bo2 (outT [128 o, 8, b]) ----
        o_ps = ps_m.tile([128, 8, BPC], FP32, tag="m", name="o_ps")
        for ob in range(8):
            for t2 in range(8):
                nc.tensor.matmul(
                    o_ps[:, ob, :],
                    wot_sb[:, t2, ob * 128 : (ob + 1) * 128],
                    attn_lhsT[t2],
                    start=(t2 == 0),
                    stop=(t2 == 7),
                )
        out_sb = consts.tile([128, 8, BPC], FP32, tag="out")
        for ob in range(8):
            nc.vector.tensor_add(out_sb[:, ob, :], o_ps[:, ob, :], bo22_sb[:, ob, :])
        nc.sync.dma_start(out=out_ext[:, :, :], in_=out_sb)

    if not nc.is_finalized():
        nc.finalize()
    return nc


def _prep_in_maps(inputs):
    query = np.asarray(inputs["query"], np.float32)
    key = np.asarray(inputs["key"], np.float32)
    value = np.asarray(inputs["value"], np.float32)
    Wq = np.asarray(inputs["Wq"], np.float32)
    bq = np.asarray(inputs["bq"], np.float32)
    Wk = np.asarray(inputs["Wk"], np.float32)
    Wv = np.asarray(inputs["Wv"], np.float32)
    Wo = np.asarray(inputs["Wo"], np.float32)
    bv = np.asarray(inputs["bv"], np.float32)
    bo = np.asarray(inputs["bo"], np.float32)

    bo2 = bo + bv @ Wo.T  # Sum of softmax weights = 1 folds bv through Wo
    shared = {
        "WqT64": np.ascontiguousarray(Wq.T * C).astype(E3),
        "Wk64": np.ascontiguousarray(Wk * C).astype(E3),
        "WvT64": np.ascontiguousarray(Wv.T * C).astype(E3),
        "WoT64": np.ascontiguousarray(Wo.T * C).astype(E3),
    }
    bqs_pack = (bq * SCALE).reshape(8, 128).T[:, :, None]  # [128, 8, 1]
    bo2_pack = bo2.reshape(8, 128).T[:, :, None]
    in_maps = []
    for c in range(NCORES):
        c0 = c * BPC
        qt = query[c0 : c0 + BPC, 0, :].T * (SCALE / C)  # [D, BPC] fp32
        qt_pack = qt.reshape(8, 128, BPC).transpose(1, 0, 2)  # [128, 8, 2]
        smalls = np.concatenate(
            [
                qt_pack,
                np.broadcast_to(bqs_pack, (128, 8, BPC)),
                np.broadcast_to(bo2_pack, (128, 8, BPC)),
            ],
            axis=2,
        )
        in_maps.append(
            {
                "smalls": np.ascontiguousarray(smalls, np.float32),
                "Kp": np.ascontiguousarray(
                    key[c0 : c0 + BPC]
                    .astype(E3)
                    .reshape(BPC, NKC, 128, 8, 128)
                    .transpose(0, 1, 4, 3, 2)
                ),
                "Vn": np.ascontiguousarray(value[c0 : c0 + BPC].astype(E3)),
                **shared,
            }
        )
    return in_maps


def kernel(**inputs):
    if "nc" not in _CACHE:
        _CACHE["nc"] = build_nc()
    nc = _CACHE["nc"]
    in_maps = _prep_in_maps(inputs)
    res = run_bass_kernel_spmd(nc, in_maps, list(range(NCORES)))
    # device gives outT [128 p, 8 n, BPC b]; full[b, n*128+p] = outT[p, n, b]
    return np.concatenate(
        [
            res.results[i]["out"].transpose(2, 1, 0).reshape(BPC, D)
            for i in range(NCORES)
        ],
        axis=0,
    )


if __name__ == "__main__":
    nc = build_nc()
    print("built ok")


# revision 24
# speedup vs baseline: 3.3504x; 1.1147x over previous
"""Trainium2 Bass kernel for nn_CrossAttention (B=16, SQ=1, SKV=4096, D=1024, H=16).

Strategy
--------
Data-parallel over batch: each of the 8 cores owns 2 batch elements.

Because SQ == 1 the K/V projections fold away:

  scores[h,kpos] = t[h,:] . key[kpos,:],   t = SCALE * blockdiag(qh) @ Wk
    (bk is constant along kpos -> cancels in softmax)
  wv[h,:]   = e[h,:] @ value            (raw value, project after)
  attn[h,:] = (wv[h,:]/S) @ Wv_h^T      (+ bv folds into bo on host: Sum w = 1)

This drops compute ~64x vs the naive form and makes the kernel DMA-bound
on streaming K and V once.  K/V and all four weights are cast to fp8
e3m4 on the host (weights x64-scaled to dodge e3m4 subnormals; the two
1/64 factors for Wv/Wo fold into the 1/S broadcast constant), halving
DMA vs bf16.  K-chunk DMAs ride the SP HWDGE queue and V-chunk/weight
DMAs the ACT queue so per-DMA setup latencies overlap.  Measured
end-to-end error vs the fp32 reference: ~6e-3 Frobenius-relative.

All matmuls run in the "flipped" orientation: the large streamed tensor
(K chunk / V chunk / weight block) is the stationary 128x128 lhsT so the
PE array is fully utilized and outputs come out pre-transposed - no PE
transposes anywhere.  Softmax skips max-subtraction (|scores| < ~0.5 by
construction); the sum S is taken with a ones-vector matmul and 1/S is
broadcast across partitions with a rank-1 fp32 matmul.

Per-core pipeline (b = 0,1 local batches):
  qT  = WqT64-blocks^T @ (queryT * SCALE/64)  + bq*SCALE      [128,8,2]
  tT  = Wk64-blocks^T @ blockdiag-masked qT                   [128,8,32]
  per b, per kpos-chunk kc (128 wide):
      scT[kc]  = Kp-tile^T @ tT(b)        (8 j-chunk accumulate)
      e[kc]    = exp(scT/64)              (ACT, bf16)
      S       += ones^T @ e[kc]           wv[jb] += V-tile^T @ e[kc]
  wvn[jb] = wv[jb] * bcast(1/S);  attn-pair[t2] = WvT-block^T @ wvn cols
  outT = WoT-blocks^T @ attn-pairs + bo2  (bo2 = bo + bv@Wo^T, host)
"""

import numpy as np
import ml_dtypes
from contextlib import ExitStack

import concourse.bass as bass
from concourse import bacc
import concourse.mybir as mybir
from concourse.tile import TileContext
from concourse.bass_utils import run_bass_kernel_spmd

B, SKV, D, H, HD = 16, 4096, 1024, 16, 64
NCORES = 8
BPC = B // NCORES  # 2 batches per core
SCALE = 1.0 / float(D) ** 0.5
C = 64.0  # fp8 pre-scale on Wq/Wk, undone in the exp()
NKC = SKV // 128  # 32 kpos chunks per batch
G = 16  # kpos chunks per K/V DMA group
NG = NKC // G

FP32 = mybir.dt.float32
BF16 = mybir.dt.bfloat16
FP8 = mybir.dt.float8e3
EXP = mybir.ActivationFunctionType.Exp

BF = np.dtype(ml_dtypes.bfloat16)
E3 = np.dtype(ml_dtypes.float8_e3m4)

_CACHE = {}


def build_nc():
    nc = bacc.Bacc("TRN2")

    # ---- kernel parameters (per core) ----
    # smalls packs qts (cols 0:2), bq*SCALE (2:4), bo2 (4:6) into one DMA
    smalls = nc.declare_dram_parameter("smalls", [128, 8, 6], FP32, isOutput=False)
    Kp = nc.declare_dram_parameter("Kp", [BPC, NKC, 128, 8, 128], FP8, isOutput=False)
    Vn = nc.declare_dram_parameter("Vn", [BPC, SKV, D], FP8, isOutput=False)
    WqT64 = nc.declare_dram_parameter("WqT64", [D, D], FP8, isOutput=False)
    Wk64 = nc.declare_dram_parameter("Wk64", [D, D], FP8, isOutput=False)
    WvT64 = nc.declare_dram_parameter("WvT64", [D, D], FP8, isOutput=False)
    WoT64 = nc.declare_dram_parameter("WoT64", [D, D], FP8, isOutput=False)
    out_ext = nc.declare_dram_parameter("out", [128, 8, BPC], FP32, isOutput=True)

    # [p, n, x] views (row r = n*128 + p)
    WqT_r = WqT64.rearrange("(n p) o -> p n o", p=128)
    Wk_r = Wk64.rearrange("(n p) o -> p n o", p=128)
    WvT_r = WvT64.rearrange("(n p) o -> p n o", p=128)
    WoT_r = WoT64.rearrange("(n p) o -> p n o", p=128)

    with TileContext(nc) as tc, ExitStack() as ctx:
        consts = ctx.enter_context(tc.tile_pool(name="consts", bufs=1))
        wqk = ctx.enter_context(tc.tile_pool(name="wqk", bufs=2))
        kp = ctx.enter_context(tc.tile_pool(name="kp", bufs=3))
        vp = ctx.enter_context(tc.tile_pool(name="vp", bufs=2))
        elp = ctx.enter_context(tc.tile_pool(name="elp", bufs=2))
        wvnp = ctx.enter_context(tc.tile_pool(name="wvnp", bufs=16))
        small = ctx.enter_context(tc.tile_pool(name="small", bufs=4))
        ps_m = ctx.enter_context(tc.tile_pool(name="ps_m", bufs=2, space="PSUM"))
        ps_sc = ctx.enter_context(tc.tile_pool(name="ps_sc", bufs=2, space="PSUM"))
        ps_wv = ctx.enter_context(tc.tile_pool(name="ps_wv", bufs=2, space="PSUM"))
        ps_s = ctx.enter_context(tc.tile_pool(name="ps_s", bufs=1, space="PSUM"))

        # ---- weight + small-constant DMAs, alternating the SP and ACT
        # HWDGE queues so per-DMA setup latencies overlap ----
        wq_sb = wqk.tile([128, 8, D], FP8, tag="w", name="wq_sb")
        nc.sync.dma_start(out=wq_sb, in_=WqT_r)
        smalls_sb = consts.tile([128, 8, 6], FP32, tag="smalls")
        nc.scalar.dma_start(out=smalls_sb, in_=smalls[:, :, :])
        wk_sb = wqk.tile([128, 8, D], FP8, tag="w", name="wk_sb")
        nc.scalar.dma_start(out=wk_sb, in_=Wk_r)
        wvt_sb = consts.tile([128, 8, D], FP8, tag="wvt")
        nc.sync.dma_start(out=wvt_sb, in_=WvT_r)
        wot_sb = consts.tile([128, 8, D], FP8, tag="wot")
        nc.scalar.dma_start(out=wot_sb, in_=WoT_r)

        bqs2_sb = smalls_sb[:, :, 2:4]
        bo22_sb = smalls_sb[:, :, 4:6]
        qts_sb = consts.tile([128, 8, BPC], BF16, tag="qts")
        nc.vector.tensor_copy(qts_sb, smalls_sb[:, :, 0:2])
        ones128 = consts.tile([128, 1], BF16, tag="ones128")
        nc.vector.memset(ones128, 1.0)
        # 1/(C*C) undoes the x64 pre-scale on both WvT64 and WoT64
        ones1 = consts.tile([1, 128], FP32, tag="ones1")
        nc.vector.memset(ones1, 1.0 / (C * C))
        qmask = consts.tile([128, 8, 32], BF16, tag="qmask")
        nc.vector.memset(qmask, 0.0)

        # db outer / ic inner: one open PSUM accumulation group per bank
        q_ps = ps_m.tile([128, 8, BPC], FP32, tag="m", name="q_ps")
        for db in range(8):
            for ic in range(8):
                nc.tensor.matmul(
                    q_ps[:, db, :],
                    wq_sb[:, ic, db * 128 : (db + 1) * 128],
                    qts_sb[:, ic, :],
                    start=(ic == 0),
                    stop=(ic == 7),
                )
        qt_sb = consts.tile([128, 8, BPC], BF16, tag="qt")
        for db in range(8):
            nc.vector.tensor_add(qt_sb[:, db, :], q_ps[:, db, :], bqs2_sb[:, db, :])

        # blockdiag mask: col 2h+b holds qT of head h (h = 2*ic + (p>=64))
        for ic in range(8):
            for b in range(BPC):
                nc.vector.tensor_copy(
                    qmask[0:64, ic, 4 * ic + b : 4 * ic + b + 1],
                    qt_sb[0:64, ic, b : b + 1],
                )
                nc.vector.tensor_copy(
                    qmask[64:128, ic, 4 * ic + 2 + b : 4 * ic + 3 + b],
                    qt_sb[64:128, ic, b : b + 1],
                )

        # ---- tT = Wk-blocks^T @ qmask (out [128 j, 8, 32(2h+b)]) ----
        t_ps = ps_m.tile([128, 8, 32], FP32, tag="m", name="t_ps")
        for jb in range(8):
            for ic in range(8):
                nc.tensor.matmul(
                    t_ps[:, jb, :],
                    wk_sb[:, ic, jb * 128 : (jb + 1) * 128],
                    qmask[:, ic, :],
                    start=(ic == 0),
                    stop=(ic == 7),
                )
        tT = [[None] * 8 for _ in range(BPC)]
        for jb in range(8):
            tv = t_ps[:, jb, :].rearrange("p (h b) -> p b h", b=BPC)
            for b in range(BPC):
                tt = consts.tile([128, 16], BF16, tag=f"tT{jb}_{b}", name=f"tT{jb}_{b}")
                nc.vector.tensor_copy(tt, tv[:, b, :])
                tT[b][jb] = tt

        # ---- per-batch attention ----
        attn_lhsT = [
            consts.tile([128, BPC], BF16, tag=f"al{t2}", name=f"al{t2}")
            for t2 in range(8)
        ]
        for b in range(BPC):
            Kp_r = Kp[b].rearrange("kc p jc k -> p kc jc k")
            Vn_r = Vn[b].rearrange("(kc p) j -> p kc j", p=128)
            e_sb = elp.tile([128, NKC, 16], BF16, tag="e")
            S_ps = ps_s.tile([1, 16], FP32, tag="s")
            vres = vp.tile([128, NKC, D], FP8, tag="v", name="vres")
            for g in range(NG):
                kg = kp.tile([128, G, 8, 128], FP8, tag="k", name="kg")
                nc.sync.dma_start(out=kg, in_=Kp_r[:, g * G : (g + 1) * G, :, :])
                nc.scalar.dma_start(
                    out=vres[:, g * G : (g + 1) * G, :],
                    in_=Vn_r[:, g * G : (g + 1) * G, :],
                )
                for c in range(G):
                    kc = g * G + c
                    sc = ps_sc.tile([128, 16], FP32, tag="sc")
                    for jc in range(8):
                        nc.tensor.matmul(
                            sc,
                            kg[:, c, jc, :],
                            tT[b][jc],
                            start=(jc == 0),
                            stop=(jc == 7),
                        )
                    nc.scalar.activation(
                        out=e_sb[:, kc, :], in_=sc, func=EXP, bias=0.0, scale=1.0 / C
                    )
            # S after the kc loop so it never head-of-line-blocks the PE
            for kc in range(NKC):
                nc.tensor.matmul(
                    S_ps,
                    ones128,
                    e_sb[:, kc, :],
                    start=(kc == 0),
                    stop=(kc == NKC - 1),
                )
            # wv pass: V is resident, one accumulation group (bank) at a time
            wv_ps = ps_wv.tile([128, 8, 16], FP32, tag="wv")
            for jb in range(8):
                for kc in range(NKC):
                    nc.tensor.matmul(
                        wv_ps[:, jb, :],
                        vres[:, kc, jb * 128 : (jb + 1) * 128],
                        e_sb[:, kc, :],
                        start=(kc == 0),
                        stop=(kc == NKC - 1),
                    )

            # 1/S broadcast across partitions (rank-1 fp32 matmul)
            rS_sb = small.tile([1, 16], FP32, tag="rs", name="rS_sb")
            nc.vector.reciprocal(rS_sb, S_ps)
            Sb_ps = ps_m.tile([128, 16], FP32, tag="m", name="Sb_ps")
            nc.tensor.matmul(Sb_ps, ones1, rS_sb, start=True, stop=True)
            Sb_sb = small.tile([128, 16], FP32, tag="sb", name="Sb_sb")
            nc.vector.tensor_copy(Sb_sb, Sb_ps)

            wvn = []
            for jb in range(8):
                w1 = wvnp.tile([128, 16], BF16, tag="wvn", name="w1")
                nc.vector.tensor_mul(w1, wv_ps[:, jb, :], Sb_sb)
                wvn.append(w1)

            # attn pairs: at2 [128(hd pair), 2(head)] per t2; diagonal extract
            for t2 in range(8):
                at2 = ps_m.tile([128, BPC], FP32, tag="m", name="at2")
                for jc in range(8):
                    nc.tensor.matmul(
                        at2,
                        wvt_sb[:, jc, t2 * 128 : (t2 + 1) * 128],
                        wvn[jc][:, 2 * t2 : 2 * t2 + 2],
                        start=(jc == 0),
                        stop=(jc == 7),
                    )
                nc.scalar.copy(attn_lhsT[t2][0:64, b : b + 1], at2[0:64, 0:1])
                nc.scalar.copy(attn_lhsT[t2][64:128, b : b + 1], at2[64:128, 1:2])

        # ---- out = WoT-blocks^T @ attn + bo2 (outT [128 o, 8, b]) ----
        o_ps = ps_m.tile([128, 8, BPC], FP32, tag="m", name="o_ps")
        for ob in range(8):
            for t2 in range(8):
                nc.tensor.matmul(
                    o_ps[:, ob, :],
                    wot_sb[:, t2, ob * 128 : (ob + 1) * 128],
                    attn_lhsT[t2],
                    start=(t2 == 0),
                    stop=(t2 == 7),
                )
        # split the result DMA across both queues so its fixed latency overlaps
        out_sb = consts.tile([128, 8, BPC], FP32, tag="out")
        for ob in range(8):
            nc.vector.tensor_add(out_sb[:, ob, :], o_ps[:, ob, :], bo22_sb[:, ob, :])
            if ob == 3:
                nc.scalar.dma_start(
                    out=out_ext[:, 0:4, :], in_=out_sb[:, 0:4, :]
                )
        nc.sync.dma_start(out=out_ext[:, 4:8, :], in_=out_sb[:, 4:8, :])

    if not nc.is_finalized():
        nc.finalize()
    return nc


def _prep_in_maps(inputs):
    query = np.asarray(inputs["query"], np.float32)
    key = np.asarray(inputs["key"], np.float32)
    value = np.asarray(inputs["value"], np.float32)
    Wq = np.asarray(inputs["Wq"], np.float32)
    bq = np.asarray(inputs["bq"], np.float32)
    Wk = np.asarray(inputs["Wk"], np.float32)
    Wv = np.asarray(inputs["Wv"], np.float32)
    Wo = np.asarray(inputs["Wo"], np.float32)
    bv = np.asarray(inputs["bv"], np.float32)
    bo = np.asarray(inputs["bo"], np.float32)

    bo2 = bo + bv @ Wo.T  # Sum of softmax weights = 1 folds bv through Wo
    shared = {
        "WqT64": np.ascontiguousarray(Wq.T * C).astype(E3),
        "Wk64": np.ascontiguousarray(Wk * C).astype(E3),
        "WvT64": np.ascontiguousarray(Wv.T * C).astype(E3),
        "WoT64": np.ascontiguousarray(Wo.T * C).astype(E3),
    }
    bqs_pack = (bq * SCALE).reshape(8, 128).T[:, :, None]  # [128, 8, 1]
    bo2_pack = bo2.reshape(8, 128).T[:, :, None]
    in_maps = []
    for c in range(NCORES):
        c0 = c * BPC
        qt = query[c0 : c0 + BPC, 0, :].T * (SCALE / C)  # [D, BPC] fp32
        qt_pack = qt.reshape(8, 128, BPC).transpose(1, 0, 2)  # [128, 8, 2]
        smalls = np.concatenate(
            [
                qt_pack,
                np.broadcast_to(bqs_pack, (128, 8, BPC)),
                np.broadcast_to(bo2_pack, (128, 8, BPC)),
            ],
            axis=2,
        )
        in_maps.append(
            {
                "smalls": np.ascontiguousarray(smalls, np.float32),
                "Kp": np.ascontiguousarray(
                    key[c0 : c0 + BPC]
                    .astype(E3)
                    .reshape(BPC, NKC, 128, 8, 128)
                    .transpose(0, 1, 4, 3, 2)
                ),
                "Vn": np.ascontiguousarray(value[c0 : c0 + BPC].astype(E3)),
                **shared,
            }
        )
    return in_maps


def kernel(**inputs):
    if "nc" not in _CACHE:
        _CACHE["nc"] = build_nc()
    nc = _CACHE["nc"]
    in_maps = _prep_in_maps(inputs)
    res = run_bass_kernel_spmd(nc, in_maps, list(range(NCORES)))
    # device gives outT [128 p, 8 n, BPC b]; full[b, n*128+p] = outT[p, n, b]
    return np.concatenate(
        [
            res.results[i]["out"].transpose(2, 1, 0).reshape(BPC, D)
            for i in range(NCORES)
        ],
        axis=0,
    )


if __name__ == "__main__":
    nc = build_nc()
    print("built ok")


# revision 28
# speedup vs baseline: 4.0836x; 1.2189x over previous
"""Trainium2 Bass kernel for nn_CrossAttention (B=16, SQ=1, SKV=4096, D=1024, H=16).

Strategy
--------
Data-parallel over batch: each of the 8 cores owns 2 batch elements.

Because SQ == 1 the K/V projections fold away:

  scores[h,kpos] = t[h,:] . key[kpos,:],   t = SCALE * blockdiag(qh) @ Wk
    (bk is constant along kpos -> cancels in softmax)
  wv[h,:]   = e[h,:] @ value            (raw value, project after)
  attn[h,:] = (wv[h,:]/S) @ Wv_h^T      (+ bv folds into bo on host: Sum w = 1)

This drops compute ~64x vs the naive form and makes the kernel DMA-bound
on streaming K and V once.  K/V and all four weights are cast to fp8
e3m4 on the host (weights x64-scaled to dodge e3m4 subnormals; the two
1/64 factors for Wv/Wo fold into the 1/S broadcast constant), halving
DMA vs bf16.  K-chunk DMAs ride the SP HWDGE queue and V-chunk/weight
DMAs the ACT queue so per-DMA setup latencies overlap.  Measured
end-to-end error vs the fp32 reference: ~6e-3 Frobenius-relative.

All matmuls run in the "flipped" orientation: the large streamed tensor
(K chunk / V chunk / weight block) is the stationary 128x128 lhsT so the
PE array is fully utilized and outputs come out pre-transposed - no PE
transposes anywhere.  Softmax skips max-subtraction (|scores| < ~0.5 by
construction); the sum S is taken with a ones-vector matmul and 1/S is
broadcast across partitions with a rank-1 fp32 matmul.

Per-core pipeline (b = 0,1 local batches):
  qT  = WqT64-blocks^T @ (queryT * SCALE/64)  + bq*SCALE      [128,8,2]
  tT  = Wk64-blocks^T @ blockdiag-masked qT                   [128,8,32]
  per b, per kpos-chunk kc (128 wide):
      scT[kc]  = Kp-tile^T @ tT(b)        (8 j-chunk accumulate)
      e[kc]    = exp(scT/64)              (ACT, bf16)
      S       += ones^T @ e[kc]           wv[jb] += V-tile^T @ e[kc]
  wvn[jb] = wv[jb] * bcast(1/S);  attn-pair[t2] = WvT-block^T @ wvn cols
  outT = WoT-blocks^T @ attn-pairs + bo2  (bo2 = bo + bv@Wo^T, host)
"""

import numpy as np
import ml_dtypes
from contextlib import ExitStack

import concourse.bass as bass
from concourse import bacc
import concourse.mybir as mybir
from concourse.tile import TileContext
from concourse.bass_utils import run_bass_kernel_spmd

B, SKV, D, H, HD = 16, 4096, 1024, 16, 64
NCORES = 8
BPC = B // NCORES  # 2 batches per core
SCALE = 1.0 / float(D) ** 0.5
C = 64.0  # fp8 pre-scale on Wq/Wk, undone in the exp()
NKC = SKV // 128  # 32 kpos chunks per batch
G = 8  # kpos chunks per K/V DMA group
NG = NKC // G

FP32 = mybir.dt.float32
BF16 = mybir.dt.bfloat16
FP8 = mybir.dt.float8e3
EXP = mybir.ActivationFunctionType.Exp

BF = np.dtype(ml_dtypes.bfloat16)
E3 = np.dtype(ml_dtypes.float8_e3m4)

_CACHE = {}


def build_nc():
    nc = bacc.Bacc("TRN2")

    # ---- kernel parameters (per core) ----
    # smalls packs qts (cols 0:2), bq*SCALE (2:4), bo2 (4:6) into one DMA
    smalls = nc.declare_dram_parameter("smalls", [128, 8, 6], FP32, isOutput=False)
    Kp = nc.declare_dram_parameter("Kp", [BPC, NKC, 128, 8, 128], FP8, isOutput=False)
    Vn = nc.declare_dram_parameter("Vn", [BPC, SKV, D], FP8, isOutput=False)
    WqT64 = nc.declare_dram_parameter("WqT64", [D, D], FP8, isOutput=False)
    Wk64 = nc.declare_dram_parameter("Wk64", [D, D], FP8, isOutput=False)
    WvT64 = nc.declare_dram_parameter("WvT64", [D, D], FP8, isOutput=False)
    WoT64 = nc.declare_dram_parameter("WoT64", [D, D], FP8, isOutput=False)
    out_ext = nc.declare_dram_parameter("out", [128, 8, BPC], FP32, isOutput=True)

    # [p, n, x] views (row r = n*128 + p)
    WqT_r = WqT64.rearrange("(n p) o -> p n o", p=128)
    Wk_r = Wk64.rearrange("(n p) o -> p n o", p=128)
    WvT_r = WvT64.rearrange("(n p) o -> p n o", p=128)
    WoT_r = WoT64.rearrange("(n p) o -> p n o", p=128)

    with TileContext(nc) as tc, ExitStack() as ctx:
        consts = ctx.enter_context(tc.tile_pool(name="consts", bufs=1))
        wqk = ctx.enter_context(tc.tile_pool(name="wqk", bufs=2))
        kp = ctx.enter_context(tc.tile_pool(name="kp", bufs=4))
        vp = ctx.enter_context(tc.tile_pool(name="vp", bufs=2))
        elp = ctx.enter_context(tc.tile_pool(name="elp", bufs=2))
        wvnp = ctx.enter_context(tc.tile_pool(name="wvnp", bufs=16))
        small = ctx.enter_context(tc.tile_pool(name="small", bufs=4))
        ps_m = ctx.enter_context(tc.tile_pool(name="ps_m", bufs=2, space="PSUM"))
        ps_sc = ctx.enter_context(tc.tile_pool(name="ps_sc", bufs=2, space="PSUM"))
        ps_wv = ctx.enter_context(tc.tile_pool(name="ps_wv", bufs=2, space="PSUM"))
        ps_s = ctx.enter_context(tc.tile_pool(name="ps_s", bufs=1, space="PSUM"))

        # ---- DMA schedule: 3 parallel queues (SP / ACT / GPSIMD) with all
        # dma_starts hoisted so compute stalls never delay DMA issue ----
        DQ = [nc.sync, nc.scalar, nc.gpsimd]
        wq_sb = wqk.tile([128, 8, D], FP8, tag="w", name="wq_sb")
        nc.sync.dma_start(out=wq_sb, in_=WqT_r)
        wk_sb = wqk.tile([128, 8, D], FP8, tag="w", name="wk_sb")
        nc.scalar.dma_start(out=wk_sb, in_=Wk_r)
        smalls_sb = consts.tile([128, 8, 6], FP32, tag="smalls")
        nc.gpsimd.dma_start(out=smalls_sb, in_=smalls[:, :, :])
        qi = 3
        kgs = [[None] * NG for _ in range(BPC)]
        vress = []
        wvt_sb = wot_sb = None
        for b in range(BPC):
            Kp_r = Kp[b].rearrange("kc p jc k -> p kc jc k")
            Vn_r = Vn[b].rearrange("(kc p) j -> p kc j", p=128)
            vres = vp.tile([128, NKC, D], FP8, tag="v", name="vres")
            vress.append(vres)
            for g in range(NG):
                kg = kp.tile([128, G, 8, 128], FP8, tag="k", name="kg")
                DQ[qi % 3].dma_start(out=kg, in_=Kp_r[:, g * G : (g + 1) * G, :, :])
                qi += 1
                kgs[b][g] = kg
                DQ[qi % 3].dma_start(
                    out=vres[:, g * G : (g + 1) * G, :],
                    in_=Vn_r[:, g * G : (g + 1) * G, :],
                )
                qi += 1
            if b == 0:
                wvt_sb = consts.tile([128, 8, D], FP8, tag="wvt")
                DQ[qi % 3].dma_start(out=wvt_sb, in_=WvT_r)
                qi += 1
                wot_sb = consts.tile([128, 8, D], FP8, tag="wot")
                DQ[qi % 3].dma_start(out=wot_sb, in_=WoT_r)
                qi += 1

        bqs2_sb = smalls_sb[:, :, 2:4]
        bo22_sb = smalls_sb[:, :, 4:6]
        qts_sb = consts.tile([128, 8, BPC], BF16, tag="qts")
        nc.vector.tensor_copy(qts_sb, smalls_sb[:, :, 0:2])
        ones128 = consts.tile([128, 1], BF16, tag="ones128")
        nc.vector.memset(ones128, 1.0)
        # 1/(C*C) undoes the x64 pre-scale on both WvT64 and WoT64
        ones1 = consts.tile([1, 128], FP32, tag="ones1")
        nc.vector.memset(ones1, 1.0 / (C * C))
        qmask = consts.tile([128, 8, 32], BF16, tag="qmask")
        nc.vector.memset(qmask, 0.0)

        # db outer / ic inner: one open PSUM accumulation group per bank
        q_ps = ps_m.tile([128, 8, BPC], FP32, tag="m", name="q_ps")
        for db in range(8):
            for ic in range(8):
                nc.tensor.matmul(
                    q_ps[:, db, :],
                    wq_sb[:, ic, db * 128 : (db + 1) * 128],
                    qts_sb[:, ic, :],
                    start=(ic == 0),
                    stop=(ic == 7),
                )
        qt_sb = consts.tile([128, 8, BPC], BF16, tag="qt")
        for db in range(8):
            nc.vector.tensor_add(qt_sb[:, db, :], q_ps[:, db, :], bqs2_sb[:, db, :])

        # blockdiag mask: col 2h+b holds qT of head h (h = 2*ic + (p>=64))
        for ic in range(8):
            for b in range(BPC):
                nc.vector.tensor_copy(
                    qmask[0:64, ic, 4 * ic + b : 4 * ic + b + 1],
                    qt_sb[0:64, ic, b : b + 1],
                )
                nc.vector.tensor_copy(
                    qmask[64:128, ic, 4 * ic + 2 + b : 4 * ic + 3 + b],
                    qt_sb[64:128, ic, b : b + 1],
                )

        # ---- tT = Wk-blocks^T @ qmask (out [128 j, 8, 32(2h+b)]) ----
        t_ps = ps_m.tile([128, 8, 32], FP32, tag="m", name="t_ps")
        for jb in range(8):
            for ic in range(8):
                nc.tensor.matmul(
                    t_ps[:, jb, :],
                    wk_sb[:, ic, jb * 128 : (jb + 1) * 128],
                    qmask[:, ic, :],
                    start=(ic == 0),
                    stop=(ic == 7),
                )
        tT = [[None] * 8 for _ in range(BPC)]
        for jb in range(8):
            tv = t_ps[:, jb, :].rearrange("p (h b) -> p b h", b=BPC)
            for b in range(BPC):
                tt = consts.tile([128, 16], BF16, tag=f"tT{jb}_{b}", name=f"tT{jb}_{b}")
                nc.vector.tensor_copy(tt, tv[:, b, :])
                tT[b][jb] = tt

        # ---- per-batch attention ----
        attn_lhsT = [
            consts.tile([128, BPC], BF16, tag=f"al{t2}", name=f"al{t2}")
            for t2 in range(8)
        ]
        for b in range(BPC):
            e_sb = elp.tile([128, NKC, 16], BF16, tag="e")
            S_ps = ps_s.tile([1, 16], FP32, tag="s")
            vres = vress[b]
            for kc in range(NKC):
                g, c = kc // G, kc % G
                sc = ps_sc.tile([128, 16], FP32, tag="sc")
                for jc in range(8):
                    nc.tensor.matmul(
                        sc,
                        kgs[b][g][:, c, jc, :],
                        tT[b][jc],
                        start=(jc == 0),
                        stop=(jc == 7),
                    )
                nc.scalar.activation(
                    out=e_sb[:, kc, :], in_=sc, func=EXP, bias=0.0, scale=1.0 / C
                )
            # S after the kc loop so it never head-of-line-blocks the PE
            for kc in range(NKC):
                nc.tensor.matmul(
                    S_ps,
                    ones128,
                    e_sb[:, kc, :],
                    start=(kc == 0),
                    stop=(kc == NKC - 1),
                )
            # wv pass: V is resident, one accumulation group (bank) at a time
            wv_ps = ps_wv.tile([128, 8, 16], FP32, tag="wv")
            for jb in range(8):
                for kc in range(NKC):
                    nc.tensor.matmul(
                        wv_ps[:, jb, :],
                        vres[:, kc, jb * 128 : (jb + 1) * 128],
                        e_sb[:, kc, :],
                        start=(kc == 0),
                        stop=(kc == NKC - 1),
                    )

            # 1/S broadcast across partitions (rank-1 fp32 matmul)
            rS_sb = small.tile([1, 16], FP32, tag="rs", name="rS_sb")
            nc.vector.reciprocal(rS_sb, S_ps)
            Sb_ps = ps_m.tile([128, 16], FP32, tag="m", name="Sb_ps")
            nc.tensor.matmul(Sb_ps, ones1, rS_sb, start=True, stop=True)
            Sb_sb = small.tile([128, 16], FP32, tag="sb", name="Sb_sb")
            nc.vector.tensor_copy(Sb_sb, Sb_ps)

            wvn = []
            for jb in range(8):
                w1 = wvnp.tile([128, 16], BF16, tag="wvn", name="w1")
                nc.vector.tensor_mul(w1, wv_ps[:, jb, :], Sb_sb)
                wvn.append(w1)

            # attn pairs: at2 [128(hd pair), 2(head)] per t2; diagonal extract
            for t2 in range(8):
                at2 = ps_m.tile([128, BPC], FP32, tag="m", name="at2")
                for jc in range(8):
                    nc.tensor.matmul(
                        at2,
                        wvt_sb[:, jc, t2 * 128 : (t2 + 1) * 128],
                        wvn[jc][:, 2 * t2 : 2 * t2 + 2],
                        start=(jc == 0),
                        stop=(jc == 7),
                    )
                nc.scalar.copy(attn_lhsT[t2][0:64, b : b + 1], at2[0:64, 0:1])
                nc.scalar.copy(attn_lhsT[t2][64:128, b : b + 1], at2[64:128, 1:2])

        # ---- out = WoT-blocks^T @ attn + bo2 (outT [128 o, 8, b]) ----
        o_ps = ps_m.tile([128, 8, BPC], FP32, tag="m", name="o_ps")
        for ob in range(8):
            for t2 in range(8):
                nc.tensor.matmul(
                    o_ps[:, ob, :],
                    wot_sb[:, t2, ob * 128 : (ob + 1) * 128],
                    attn_lhsT[t2],
                    start=(t2 == 0),
                    stop=(t2 == 7),
                )
        # split the result DMA across both queues so its fixed latency overlaps
        out_sb = consts.tile([128, 8, BPC], FP32, tag="out")
        for ob in range(8):
            nc.vector.tensor_add(out_sb[:, ob, :], o_ps[:, ob, :], bo22_sb[:, ob, :])
            if ob == 3:
                nc.scalar.dma_start(
                    out=out_ext[:, 0:4, :], in_=out_sb[:, 0:4, :]
                )
        nc.sync.dma_start(out=out_ext[:, 4:8, :], in_=out_sb[:, 4:8, :])

    if not nc.is_finalized():
        nc.finalize()
    return nc


def _prep_in_maps(inputs):
    query = np.asarray(inputs["query"], np.float32)
    key = np.asarray(inputs["key"], np.float32)
    value = np.asarray(inputs["value"], np.float32)
    Wq = np.asarray(inputs["Wq"], np.float32)
    bq = np.asarray(inputs["bq"], np.float32)
    Wk = np.asarray(inputs["Wk"], np.float32)
    Wv = np.asarray(inputs["Wv"], np.float32)
    Wo = np.asarray(inputs["Wo"], np.float32)
    bv = np.asarray(inputs["bv"], np.float32)
    bo = np.asarray(inputs["bo"], np.float32)

    bo2 = bo + bv @ Wo.T  # Sum of softmax weights = 1 folds bv through Wo
    shared = {
        "WqT64": np.ascontiguousarray(Wq.T * C).astype(E3),
        "Wk64": np.ascontiguousarray(Wk * C).astype(E3),
        "WvT64": np.ascontiguousarray(Wv.T * C).astype(E3),
        "WoT64": np.ascontiguousarray(Wo.T * C).astype(E3),
    }
    bqs_pack = (bq * SCALE).reshape(8, 128).T[:, :, None]  # [128, 8, 1]
    bo2_pack = bo2.reshape(8, 128).T[:, :, None]
    in_maps = []
    for c in range(NCORES):
        c0 = c * BPC
        qt = query[c0 : c0 + BPC, 0, :].T * (SCALE / C)  # [D, BPC] fp32
        qt_pack = qt.reshape(8, 128, BPC).transpose(1, 0, 2)  # [128, 8, 2]
        smalls = np.concatenate(
            [
                qt_pack,
                np.broadcast_to(bqs_pack, (128, 8, BPC)),
                np.broadcast_to(bo2_pack, (128, 8, BPC)),
            ],
            axis=2,
        )
        in_maps.append(
            {
                "smalls": np.ascontiguousarray(smalls, np.float32),
                "Kp": np.ascontiguousarray(
                    key[c0 : c0 + BPC]
                    .astype(E3)
                    .reshape(BPC, NKC, 128, 8, 128)
                    .transpose(0, 1, 4, 3, 2)
                ),
                "Vn": np.ascontiguousarray(value[c0 : c0 + BPC].astype(E3)),
                **shared,
            }
        )
    return in_maps


def kernel(**inputs):
    if "nc" not in _CACHE:
        _CACHE["nc"] = build_nc()
    nc = _CACHE["nc"]
    in_maps = _prep_in_maps(inputs)
    res = run_bass_kernel_spmd(nc, in_maps, list(range(NCORES)))
    # device gives outT [128 p, 8 n, BPC b]; full[b, n*128+p] = outT[p, n, b]
    return np.concatenate(
        [
            res.results[i]["out"].transpose(2, 1, 0).reshape(BPC, D)
            for i in range(NCORES)
        ],
        axis=0,
    )


if __name__ == "__main__":
    nc = build_nc()
    print("built ok")


# revision 30
# speedup vs baseline: 4.3794x; 1.0724x over previous
"""Trainium2 Bass kernel for nn_CrossAttention (B=16, SQ=1, SKV=4096, D=1024, H=16).

Strategy
--------
Data-parallel over batch: each of the 8 cores owns 2 batch elements.

Because SQ == 1 the K/V projections fold away:

  scores[h,kpos] = t[h,:] . key[kpos,:],   t = SCALE * blockdiag(qh) @ Wk
    (bk is constant along kpos -> cancels in softmax)
  wv[h,:]   = e[h,:] @ value            (raw value, project after)
  attn[h,:] = (wv[h,:]/S) @ Wv_h^T      (+ bv folds into bo on host: Sum w = 1)

This drops compute ~64x vs the naive form and makes the kernel DMA-bound
on streaming K and V once.  K/V and all four weights are cast to fp8
e3m4 on the host (weights x64-scaled to dodge e3m4 subnormals; the two
1/64 factors for Wv/Wo fold into the 1/S broadcast constant), halving
DMA vs bf16.  K-chunk DMAs ride the SP HWDGE queue and V-chunk/weight
DMAs the ACT queue so per-DMA setup latencies overlap.  Measured
end-to-end error vs the fp32 reference: ~6e-3 Frobenius-relative.

All matmuls run in the "flipped" orientation: the large streamed tensor
(K chunk / V chunk / weight block) is the stationary 128x128 lhsT so the
PE array is fully utilized and outputs come out pre-transposed - no PE
transposes anywhere.  Softmax skips max-subtraction (|scores| < ~0.5 by
construction); the sum S is taken with a ones-vector matmul and 1/S is
broadcast across partitions with a rank-1 fp32 matmul.

Per-core pipeline (b = 0,1 local batches):
  qT  = WqT64-blocks^T @ (queryT * SCALE/64)  + bq*SCALE      [128,8,2]
  tT  = Wk64-blocks^T @ blockdiag-masked qT                   [128,8,32]
  per b, per kpos-chunk kc (128 wide):
      scT[kc]  = Kp-tile^T @ tT(b)        (8 j-chunk accumulate)
      e[kc]    = exp(scT/64)              (ACT, bf16)
      S       += ones^T @ e[kc]           wv[jb] += V-tile^T @ e[kc]
  wvn[jb] = wv[jb] * bcast(1/S);  attn-pair[t2] = WvT-block^T @ wvn cols
  outT = WoT-blocks^T @ attn-pairs + bo2  (bo2 = bo + bv@Wo^T, host)
"""

import numpy as np
import ml_dtypes
from contextlib import ExitStack

import concourse.bass as bass
from concourse import bacc
import concourse.mybir as mybir
from concourse.tile import TileContext
from concourse.bass_utils import run_bass_kernel_spmd

B, SKV, D, H, HD = 16, 4096, 1024, 16, 64
NCORES = 8
BPC = B // NCORES  # 2 batches per core
SCALE = 1.0 / float(D) ** 0.5
C = 64.0  # fp8 pre-scale on Wq/Wk, undone in the exp()
NKC = SKV // 128  # 32 kpos chunks per batch
G = 8  # kpos chunks per K/V DMA group
NG = NKC // G

FP32 = mybir.dt.float32
BF16 = mybir.dt.bfloat16
FP8 = mybir.dt.float8e3
EXP = mybir.ActivationFunctionType.Exp

BF = np.dtype(ml_dtypes.bfloat16)
E3 = np.dtype(ml_dtypes.float8_e3m4)

_CACHE = {}


def build_nc():
    nc = bacc.Bacc("TRN2")

    # ---- kernel parameters (per core) ----
    # smalls packs qts (cols 0:2), bq*SCALE (2:4), bo2 (4:6) into one DMA
    smalls = nc.declare_dram_parameter("smalls", [128, 8, 6], FP32, isOutput=False)
    Kp = nc.declare_dram_parameter("Kp", [BPC, NKC, 128, 8, 128], FP8, isOutput=False)
    Vn = nc.declare_dram_parameter("Vn", [BPC, SKV, D], FP8, isOutput=False)
    WqT64 = nc.declare_dram_parameter("WqT64", [D, D], FP8, isOutput=False)
    Wk64 = nc.declare_dram_parameter("Wk64", [D, D], FP8, isOutput=False)
    WvT64 = nc.declare_dram_parameter("WvT64", [D, D], FP8, isOutput=False)
    WoT64 = nc.declare_dram_parameter("WoT64", [D, D], FP8, isOutput=False)
    out_ext = nc.declare_dram_parameter("out", [128, 8, BPC], FP32, isOutput=True)

    # [p, n, x] views (row r = n*128 + p)
    WqT_r = WqT64.rearrange("(n p) o -> p n o", p=128)
    Wk_r = Wk64.rearrange("(n p) o -> p n o", p=128)
    WvT_r = WvT64.rearrange("(n p) o -> p n o", p=128)
    WoT_r = WoT64.rearrange("(n p) o -> p n o", p=128)

    with TileContext(nc) as tc, ExitStack() as ctx:
        consts = ctx.enter_context(tc.tile_pool(name="consts", bufs=1))
        wqk = ctx.enter_context(tc.tile_pool(name="wqk", bufs=2))
        kp = ctx.enter_context(tc.tile_pool(name="kp", bufs=8))
        vp = ctx.enter_context(tc.tile_pool(name="vp", bufs=2))
        elp = ctx.enter_context(tc.tile_pool(name="elp", bufs=2))
        wvnp = ctx.enter_context(tc.tile_pool(name="wvnp", bufs=16))
        small = ctx.enter_context(tc.tile_pool(name="small", bufs=4))
        ps_m = ctx.enter_context(tc.tile_pool(name="ps_m", bufs=2, space="PSUM"))
        ps_sc = ctx.enter_context(tc.tile_pool(name="ps_sc", bufs=2, space="PSUM"))
        ps_wv = ctx.enter_context(tc.tile_pool(name="ps_wv", bufs=2, space="PSUM"))
        ps_s = ctx.enter_context(tc.tile_pool(name="ps_s", bufs=1, space="PSUM"))

        # constants first so Pool-dispatched memsets precede SWDGE desc-gens
        ones128 = consts.tile([128, 1], BF16, tag="ones128")
        nc.vector.memset(ones128, 1.0)
        # 1/(C*C) undoes the x64 pre-scale on both WvT64 and WoT64
        ones1 = consts.tile([1, 128], FP32, tag="ones1")
        nc.vector.memset(ones1, 1.0 / (C * C))
        qmask = consts.tile([128, 8, 32], BF16, tag="qmask")
        nc.vector.memset(qmask, 0.0)

        # ---- DMA schedule: 3 parallel queues (SP / ACT / GPSIMD), all
        # dma_starts hoisted; all K chunks first (both batches) so every
        # score/exp overlaps the V stream; V-b1 last on each queue ----
        SY, SC, GP = nc.sync, nc.scalar, nc.gpsimd
        wq_sb = wqk.tile([128, 8, D], FP8, tag="w", name="wq_sb")
        GP.dma_start(out=wq_sb, in_=WqT_r)
        wk_sb = wqk.tile([128, 8, D], FP8, tag="w", name="wk_sb")
        GP.dma_start(out=wk_sb, in_=Wk_r)
        smalls_sb = consts.tile([128, 8, 6], FP32, tag="smalls")
        SY.dma_start(out=smalls_sb, in_=smalls[:, :, :])

        Kp_rs = [Kp[b].rearrange("kc p jc k -> p kc jc k") for b in range(BPC)]
        Vn_rs = [Vn[b].rearrange("(kc p) j -> p kc j", p=128) for b in range(BPC)]
        kgs = [[None] * NG for _ in range(BPC)]
        vress = [
            vp.tile([128, NKC, D], FP8, tag="v", name=f"vres{b}") for b in range(BPC)
        ]
        KENG = [SY, SC, GP, SY, SC, GP, SY, SC]
        for i, (b, g) in enumerate([(b, g) for b in range(BPC) for g in range(NG)]):
            kg = kp.tile([128, G, 8, 128], FP8, tag="k", name="kg")
            KENG[i].dma_start(out=kg, in_=Kp_rs[b][:, g * G : (g + 1) * G, :, :])
            kgs[b][g] = kg
        VENG0 = [GP, SY, SC, GP]
        for g in range(NG):
            VENG0[g].dma_start(
                out=vress[0][:, g * G : (g + 1) * G, :],
                in_=Vn_rs[0][:, g * G : (g + 1) * G, :],
            )
        wvt_sb = consts.tile([128, 8, D], FP8, tag="wvt")
        SY.dma_start(out=wvt_sb, in_=WvT_r)
        wot_sb = consts.tile([128, 8, D], FP8, tag="wot")
        SC.dma_start(out=wot_sb, in_=WoT_r)
        VENG1 = [SY, SC, GP, SY]
        for g in range(NG):
            VENG1[g].dma_start(
                out=vress[1][:, g * G : (g + 1) * G, :],
                in_=Vn_rs[1][:, g * G : (g + 1) * G, :],
            )

        bqs2_sb = smalls_sb[:, :, 2:4]
        bo22_sb = smalls_sb[:, :, 4:6]
        qts_sb = consts.tile([128, 8, BPC], BF16, tag="qts")
        nc.vector.tensor_copy(qts_sb, smalls_sb[:, :, 0:2])

        # db outer / ic inner: one open PSUM accumulation group per bank
        q_ps = ps_m.tile([128, 8, BPC], FP32, tag="m", name="q_ps")
        for db in range(8):
            for ic in range(8):
                nc.tensor.matmul(
                    q_ps[:, db, :],
                    wq_sb[:, ic, db * 128 : (db + 1) * 128],
                    qts_sb[:, ic, :],
                    start=(ic == 0),
                    stop=(ic == 7),
                )
        qt_sb = consts.tile([128, 8, BPC], BF16, tag="qt")
        for db in range(8):
            nc.vector.tensor_add(qt_sb[:, db, :], q_ps[:, db, :], bqs2_sb[:, db, :])

        # blockdiag mask: col 2h+b holds qT of head h (h = 2*ic + (p>=64))
        for ic in range(8):
            for b in range(BPC):
                nc.vector.tensor_copy(
                    qmask[0:64, ic, 4 * ic + b : 4 * ic + b + 1],
                    qt_sb[0:64, ic, b : b + 1],
                )
                nc.vector.tensor_copy(
                    qmask[64:128, ic, 4 * ic + 2 + b : 4 * ic + 3 + b],
                    qt_sb[64:128, ic, b : b + 1],
                )

        # ---- tT = Wk-blocks^T @ qmask (out [128 j, 8, 32(2h+b)]) ----
        t_ps = ps_m.tile([128, 8, 32], FP32, tag="m", name="t_ps")
        for jb in range(8):
            for ic in range(8):
                nc.tensor.matmul(
                    t_ps[:, jb, :],
                    wk_sb[:, ic, jb * 128 : (jb + 1) * 128],
                    qmask[:, ic, :],
                    start=(ic == 0),
                    stop=(ic == 7),
                )
        tT = [[None] * 8 for _ in range(BPC)]
        for jb in range(8):
            tv = t_ps[:, jb, :].rearrange("p (h b) -> p b h", b=BPC)
            for b in range(BPC):
                tt = consts.tile([128, 16], BF16, tag=f"tT{jb}_{b}", name=f"tT{jb}_{b}")
                nc.vector.tensor_copy(tt, tv[:, b, :])
                tT[b][jb] = tt

        # ---- per-batch attention ----
        attn_lhsT = [
            consts.tile([128, BPC], BF16, tag=f"al{t2}", name=f"al{t2}")
            for t2 in range(8)
        ]
        for b in range(BPC):
            e_sb = elp.tile([128, NKC, 16], BF16, tag="e")
            S_ps = ps_s.tile([1, 16], FP32, tag="s")
            vres = vress[b]
            for kc in range(NKC):
                g, c = kc // G, kc % G
                sc = ps_sc.tile([128, 16], FP32, tag="sc")
                for jc in range(8):
                    nc.tensor.matmul(
                        sc,
                        kgs[b][g][:, c, jc, :],
                        tT[b][jc],
                        start=(jc == 0),
                        stop=(jc == 7),
                    )
                nc.scalar.activation(
                    out=e_sb[:, kc, :], in_=sc, func=EXP, bias=0.0, scale=1.0 / C
                )
            # S after the kc loop so it never head-of-line-blocks the PE
            for kc in range(NKC):
                nc.tensor.matmul(
                    S_ps,
                    ones128,
                    e_sb[:, kc, :],
                    start=(kc == 0),
                    stop=(kc == NKC - 1),
                )
            # wv pass: V is resident, one accumulation group (bank) at a time
            wv_ps = ps_wv.tile([128, 8, 16], FP32, tag="wv")
            for jb in range(8):
                for kc in range(NKC):
                    nc.tensor.matmul(
                        wv_ps[:, jb, :],
                        vres[:, kc, jb * 128 : (jb + 1) * 128],
                        e_sb[:, kc, :],
                        start=(kc == 0),
                        stop=(kc == NKC - 1),
                    )

            # 1/S broadcast across partitions (rank-1 fp32 matmul)
            rS_sb = small.tile([1, 16], FP32, tag="rs", name="rS_sb")
            nc.vector.reciprocal(rS_sb, S_ps)
            Sb_ps = ps_m.tile([128, 16], FP32, tag="m", name="Sb_ps")
            nc.tensor.matmul(Sb_ps, ones1, rS_sb, start=True, stop=True)
            Sb_sb = small.tile([128, 16], FP32, tag="sb", name="Sb_sb")
            nc.vector.tensor_copy(Sb_sb, Sb_ps)

            wvn = []
            for jb in range(8):
                w1 = wvnp.tile([128, 16], BF16, tag="wvn", name="w1")
                nc.vector.tensor_mul(w1, wv_ps[:, jb, :], Sb_sb)
                wvn.append(w1)

            # attn pairs: at2 [128(hd pair), 2(head)] per t2; diagonal extract
            for t2 in range(8):
                at2 = ps_m.tile([128, BPC], FP32, tag="m", name="at2")
                for jc in range(8):
                    nc.tensor.matmul(
                        at2,
                        wvt_sb[:, jc, t2 * 128 : (t2 + 1) * 128],
                        wvn[jc][:, 2 * t2 : 2 * t2 + 2],
                        start=(jc == 0),
                        stop=(jc == 7),
                    )
                nc.scalar.copy(attn_lhsT[t2][0:64, b : b + 1], at2[0:64, 0:1])
                nc.scalar.copy(attn_lhsT[t2][64:128, b : b + 1], at2[64:128, 1:2])

        # ---- out = WoT-blocks^T @ attn + bo2 (outT [128 o, 8, b]) ----
        o_ps = ps_m.tile([128, 8, BPC], FP32, tag="m", name="o_ps")
        for ob in range(8):
            for t2 in range(8):
                nc.tensor.matmul(
                    o_ps[:, ob, :],
                    wot_sb[:, t2, ob * 128 : (ob + 1) * 128],
                    attn_lhsT[t2],
                    start=(t2 == 0),
                    stop=(t2 == 7),
                )
        # split the result DMA across both queues so its fixed latency overlaps
        out_sb = consts.tile([128, 8, BPC], FP32, tag="out")
        for ob in range(8):
            nc.vector.tensor_add(out_sb[:, ob, :], o_ps[:, ob, :], bo22_sb[:, ob, :])
            if ob == 3:
                nc.scalar.dma_start(
                    out=out_ext[:, 0:4, :], in_=out_sb[:, 0:4, :]
                )
        nc.sync.dma_start(out=out_ext[:, 4:8, :], in_=out_sb[:, 4:8, :])

    if not nc.is_finalized():
        nc.finalize()
    return nc


def _prep_in_maps(inputs):
    query = np.asarray(inputs["query"], np.float32)
    key = np.asarray(inputs["key"], np.float32)
    value = np.asarray(inputs["value"], np.float32)
    Wq = np.asarray(inputs["Wq"], np.float32)
    bq = np.asarray(inputs["bq"], np.float32)
    Wk = np.asarray(inputs["Wk"], np.float32)
    Wv = np.asarray(inputs["Wv"], np.float32)
    Wo = np.asarray(inputs["Wo"], np.float32)
    bv = np.asarray(inputs["bv"], np.float32)
    bo = np.asarray(inputs["bo"], np.float32)

    bo2 = bo + bv @ Wo.T  # Sum of softmax weights = 1 folds bv through Wo
    shared = {
        "WqT64": np.ascontiguousarray(Wq.T * C).astype(E3),
        "Wk64": np.ascontiguousarray(Wk * C).astype(E3),
        "WvT64": np.ascontiguousarray(Wv.T * C).astype(E3),
        "WoT64": np.ascontiguousarray(Wo.T * C).astype(E3),
    }
    bqs_pack = (bq * SCALE).reshape(8, 128).T[:, :, None]  # [128, 8, 1]
    bo2_pack = bo2.reshape(8, 128).T[:, :, None]
    in_maps = []
    for c in range(NCORES):
        c0 = c * BPC
        qt = query[c0 : c0 + BPC, 0, :].T * (SCALE / C)  # [D, BPC] fp32
        qt_pack = qt.reshape(8, 128, BPC).transpose(1, 0, 2)  # [128, 8, 2]
        smalls = np.concatenate(
            [
                qt_pack,
                np.broadcast_to(bqs_pack, (128, 8, BPC)),
                np.broadcast_to(bo2_pack, (128, 8, BPC)),
            ],
            axis=2,
        )
        in_maps.append(
            {
                "smalls": np.ascontiguousarray(smalls, np.float32),
                "Kp": np.ascontiguousarray(
                    key[c0 : c0 + BPC]
                    .astype(E3)
                    .reshape(BPC, NKC, 128, 8, 128)
                    .transpose(0, 1, 4, 3, 2)
                ),
                "Vn": np.ascontiguousarray(value[c0 : c0 + BPC].astype(E3)),
                **shared,
            }
        )
    return in_maps


def kernel(**inputs):
    if "nc" not in _CACHE:
        _CACHE["nc"] = build_nc()
    nc = _CACHE["nc"]
    in_maps = _prep_in_maps(inputs)
    res = run_bass_kernel_spmd(nc, in_maps, list(range(NCORES)))
    # device gives outT [128 p, 8 n, BPC b]; full[b, n*128+p] = outT[p, n, b]
    return np.concatenate(
        [
            res.results[i]["out"].transpose(2, 1, 0).reshape(BPC, D)
            for i in range(NCORES)
        ],
        axis=0,
    )


if __name__ == "__main__":
    nc = build_nc()
    print("built ok")
